# revision 1
# baseline (speedup 1.0000x reference)
"""HeightAwarePointNetTiny on 8 Trainium2 NeuronCores (Bass/Tile).

The reference LocalAggBlock computes, per point i,
    out_i = max_{j in KNN(i)} relu(W [f_i; f_j - f_i; p_j - p_i] + b).
The pre-activation separates into a j-only and an i-only part:
    u_j = W_df f_j + W_dp p_j,   v_i = (W_f - W_df) f_i - W_dp p_i + b
    out_i = relu(v_i + max_{j in KNN(i)} u_j)
so each block is two small matmuls plus a gather-max over the KNN index
lists — no k-wide MLP.  Everything runs channel-major ([C, N]) so matmul
outputs chain without transposes; neighbor gathers use GPSIMD ap_gather
over the free axis.

Sharding: core c owns cloud c//2, query half c%2 (4096 rows).  Cross-core
data: f1 (AllGather over pairs) and the global max pool (AllReduce-max).

KNN top-16 per query row: PE emits score rows s = -dist^2 into PSUM; DVE
max8/max_index/match_replace extract exact top-16 values + indices.
"""
import sys, os
sys.path.insert(0, '/opt/trn_rl_repo')
import numpy as np
from contextlib import ExitStack

import concourse.bass as bass
import concourse.tile as tile
from concourse import bacc, mybir

dt = mybir.dt
F32 = dt.float32

B, N, IN_CH = 4, 8192, 4
K = 16
W0, W1, W2 = 64, 128, 256
NUM_CLASSES = 3
NCORES = 8
P = 128
CH = 512                      # matmul free-dim chunk
HALVES = 2
SEL_CHUNK = int(os.environ.get("SEL_CHUNK", "0"))  # 0 = flat exact top-16


def build_program(n=N, ncores=NCORES, sel_chunk=SEL_CHUNK):
    nq = n // HALVES
    nt = nq // P
    nch = n // CH
    nqch = nq // CH
    nc = bacc.Bacc("TRN2", target_bir_lowering=False, debug=False,
                   num_devices=ncores)

    xT1 = nc.dram_tensor("xT1", [5, n], F32, kind="ExternalInput")
    qxT1 = nc.dram_tensor("qxT1", [5, nq], F32, kind="ExternalInput")
    wm = {}
    for name, shape in [
        ("m_coords", [5, 3]), ("stem_w", [4, W0]), ("stem_b", [W0, 1]),
        ("w1_u_a", [W0, W1]), ("w1_u_b", [3, W1]),
        ("w1_v_a", [W0, W1]), ("w1_v_b", [3, W1]), ("b1_b", [W1, 1]),
        ("w2_u_a", [W1, W2]), ("w2_u_b", [3, W2]),
        ("w2_v_a", [W1, W2]), ("w2_v_b", [3, W2]), ("b2_b", [128, 2]),
        ("glob_k0", [128, W2]), ("glob_k1", [128, W2]), ("glob_b", [128, 2]),
        ("h1a_k0", [128, W2]), ("h1a_k1", [128, W2]),
        ("h1g_k0", [128, W2]), ("h1g_k1", [128, W2]), ("h1_b", [128, 2]),
        ("h2_k0", [128, NUM_CLASSES]), ("h2_k1", [128, NUM_CLASSES]),
        ("h2_b", [NUM_CLASSES, 1]), ("sig_par", [1, 3]),
    ]:
        wm[name] = nc.dram_tensor(name, shape, F32, kind="ExternalInput")

    out_lg = nc.dram_tensor("out_lg", [NUM_CLASSES, nq], F32,
                            kind="ExternalOutput")
    coords_dram = nc.dram_tensor("coords_dram", [3, n], F32)
    f1_loc = nc.dram_tensor("f1_loc", [W1, nq], F32)
    f1_gath = nc.dram_tensor("f1_gath", [HALVES, W1, nq], F32)
    g_loc = nc.dram_tensor("g_loc", [W2, 1], F32)
    g_red = nc.dram_tensor("g_red", [W2, 1], F32)
    PAIRS = [[c, c + 1] for c in range(0, ncores, 2)] if ncores > 1 else []

    Relu = mybir.ActivationFunctionType.Relu
    Copy = mybir.ActivationFunctionType.Copy
    Sigmoid = mybir.ActivationFunctionType.Sigmoid
    Square = mybir.ActivationFunctionType.Square
    AX = mybir.AxisListType.X
    MAX = mybir.AluOpType.max
    ADD = mybir.AluOpType.add

    with tile.TileContext(nc) as tc, ExitStack() as ctx:
        pers = ctx.enter_context(tc.tile_pool(name="pers", bufs=1))
        lpool = ctx.enter_context(tc.tile_pool(name="lp", bufs=2))
        gpool = ctx.enter_context(tc.tile_pool(name="gp", bufs=1))
        stg = ctx.enter_context(tc.tile_pool(name="stg", bufs=2))
        ppool = ctx.enter_context(tc.tile_pool(name="ps", bufs=4, space="PSUM"))

        def mm_chain(dst, dst_sl, parts, act=Copy, bias=0.0, scale=1.0,
                     shape=(P, CH)):
            ps = ppool.tile(list(shape), F32, tag="mm", name="mmps")
            for ix, (lhsT, rhs) in enumerate(parts):
                nc.tensor.matmul(ps[:], lhsT, rhs, start=(ix == 0),
                                 stop=(ix == len(parts) - 1))
            nc.scalar.activation(dst[:, dst_sl], ps[:], act, bias=bias,
                                 scale=scale)

        W = {}
        for name in wm:
            t = pers.tile(list(wm[name].shape), F32, tag=name, name=name)
            nc.sync.dma_start(t[:], wm[name].ap())
            W[name] = t
        wrap_all = pers.tile([P, nt * P], dt.uint16, tag="wrap_all")
        ones3 = pers.tile([3, 1], F32, tag="ones3")
        nc.vector.memset(ones3[:], 1.0)
        cst = pers.tile([1, 2 * CH], F32, tag="cst")
        nc.vector.memset(cst[:, 0:CH], -1.0)
        nc.vector.memset(cst[:, CH:2 * CH], 1.0)

        with tc.tile_pool(name="poolC", bufs=1) as poolC:
            q5 = poolC.tile([5, nq], F32, tag="q5")
            f1T = poolC.tile([W1, nq], F32, tag="f1T")

            with tc.tile_pool(name="poolB", bufs=1) as poolB, \
                 tc.tile_pool(name="spool", bufs=1) as spool:
                rhs5 = poolB.tile([5, n], F32, tag="rhs5")
                U1T = poolB.tile([W1, n], F32, tag="U1T")

                # ---- streamed setup over candidate chunks ----
                for i in range(nch):
                    sl = bass.ts(i, CH)
                    xch = stg.tile([5, CH], F32, tag="xch")
                    nc.sync.dma_start(xch[:], xT1.ap()[:, sl])
                    cch = stg.tile([3, CH], F32, tag="cch")
                    ps = ppool.tile([3, CH], F32, tag="mm", name="csps")
                    nc.tensor.matmul(ps[:], W["m_coords"][:], xch[:],
                                     start=True, stop=True)
                    nc.scalar.activation(cch[:], ps[:], Copy, scale=1.0)
                    nc.scalar.activation(rhs5[0:3, sl], ps[:], Copy,
                                         scale=2.0)
                    nc.sync.dma_start(coords_dram.ap()[:, sl], cch[:])
                    sqs = stg.tile([3, CH], F32, tag="sqs")
                    nc.scalar.activation(sqs[:], cch[:], Square)
                    xxs = stg.tile([1, CH], F32, tag="xxs")
                    mm_chain(xxs, slice(0, CH), [(ones3[:], sqs[:])],
                             scale=-1.0, shape=(1, CH))
                    nc.sync.dma_start(rhs5[4:5, sl], xxs[:])
                    nc.sync.dma_start(rhs5[3:4, sl], cst[0:1, 0:CH])
                    f64 = stg.tile([W0, CH], F32, tag="f64")
                    mm_chain(f64, slice(0, CH),
                             [(W["stem_w"][:], xch[0:4, :])],
                             act=Relu, bias=W["stem_b"][:, 0:1],
                             shape=(W0, CH))
                    mm_chain(U1T, sl, [(W["w1_u_a"][:], f64[:]),
                                       (W["w1_u_b"][:], cch[:])])

                # ---- streamed setup over query chunks (V1 -> f1T) ----
                for i in range(nqch):
                    sl = bass.ts(i, CH)
                    xch = stg.tile([5, CH], F32, tag="xch")
                    nc.sync.dma_start(xch[:], qxT1.ap()[:, sl])
                    ps = ppool.tile([3, CH], F32, tag="mm", name="qcps")
                    nc.tensor.matmul(ps[:], W["m_coords"][:], xch[:],
                                     start=True, stop=True)
                    nc.scalar.activation(q5[0:3, sl], ps[:], Copy, scale=1.0)
                    sqs = stg.tile([3, CH], F32, tag="sqs")
                    nc.scalar.activation(sqs[:], ps[:], Square)
                    xxs = stg.tile([1, CH], F32, tag="xxs")
                    mm_chain(xxs, slice(0, CH), [(ones3[:], sqs[:])],
                             shape=(1, CH))
                    nc.sync.dma_start(q5[3:4, sl], xxs[:])
                    nc.sync.dma_start(q5[4:5, sl], cst[0:1, CH:2 * CH])
                    f64 = stg.tile([W0, CH], F32, tag="f64")
                    mm_chain(f64, slice(0, CH),
                             [(W["stem_w"][:], xch[0:4, :])],
                             act=Relu, bias=W["stem_b"][:, 0:1],
                             shape=(W0, CH))
                    qcch = stg.tile([3, CH], F32, tag="cch")
                    nc.scalar.activation(qcch[:], q5[0:3, sl], Copy,
                                         scale=1.0)
                    mm_chain(f1T, sl, [(W["w1_v_a"][:], f64[:]),
                                       (W["w1_v_b"][:], qcch[:])])

                # ---- loop 1: selection + block1 gather-max ----
                for t in range(nt):
                    tsl = bass.ts(t, P)
                    srow = spool.tile([P, n], F32, tag="srow")
                    for i in range(nch):
                        ps = ppool.tile([P, CH], F32, tag="mm", name="sps")
                        nc.tensor.matmul(ps[:], q5[:, tsl],
                                         rhs5[:, bass.ts(i, CH)],
                                         start=True, stop=True)
                        nc.scalar.activation(srow[:, bass.ts(i, CH)], ps[:],
                                             Copy, scale=1.0)
                    w16 = lpool.tile([P, K], F32, tag="w16")
                    gi = lpool.tile([P, K], dt.uint16, tag="gi")
                    if sel_chunk:
                        nsc = n // sel_chunk
                        cand = lpool.tile([P, 8 * nsc], F32, tag="cand")
                        for j in range(nsc):
                            nc.vector.max(cand[:, bass.ts(j, 8)],
                                          srow[:, bass.ts(j, sel_chunk)])
                        nc.vector.max(w16[:, 0:8], cand[:])
                        nc.vector.match_replace(cand[:], w16[:, 0:8],
                                                cand[:], -3e38)
                        nc.vector.max(w16[:, 8:16], cand[:])
                    else:
                        nc.vector.max(w16[:, 0:8], srow[:])
                        nc.vector.max_index(gi[:, 0:8], w16[:, 0:8],
                                            srow[:])
                        nc.vector.match_replace(srow[:], w16[:, 0:8],
                                                srow[:], -3e38)
                        nc.vector.max(w16[:, 8:16], srow[:])
                    if sel_chunk:
                        nc.vector.max_index(gi[:, 0:8], w16[:, 0:8],
                                            srow[:])
                    nc.vector.max_index(gi[:, 8:16], w16[:, 8:16], srow[:])

                    gip = lpool.tile([P, 32], dt.uint16, tag="gip")
                    nc.vector.memset(gip[:], 0)
                    nc.vector.tensor_copy(gip[:, 0:16], gi[:])
                    giT = lpool.tile([32, P], dt.uint16, tag="giT")
                    for b_ in range(4):
                        nc.vector.transpose(
                            giT[0:32, 32 * b_:32 * b_ + 32],
                            gip[32 * b_:32 * b_ + 32, 0:32])
                    for g in range(8):
                        nc.sync.dma_start(wrap_all[16 * g:16 * g + 16, tsl],
                                          giT[0:16, :])

                    gat = gpool.tile([P, P * K], F32, tag="gat")
                    nc.gpsimd.ap_gather(
                        gat[:].rearrange("c (n d) -> c n d", d=1),
                        U1T[:].rearrange("c (n d) -> c n d", d=1),
                        wrap_all[:, tsl].bitcast(dt.int16),
                        channels=P, num_elems=n, d=1, num_idxs=P * K)
                    h1 = lpool.tile([P, P], F32, tag="h1")
                    nc.vector.tensor_reduce(
                        h1[:], gat[:].rearrange("c (q s) -> c q s", s=K),
                        axis=AX, op=MAX)
                    nc.vector.tensor_tensor(h1[:], h1[:], f1T[:, tsl],
                                            op=ADD)
                    nc.scalar.activation(f1T[:, tsl], h1[:], Relu,
                                         bias=W["b1_b"][:, 0:1], scale=1.0)

            # ---- exchange f1 halves within the pair ----
            nc.sync.dma_start(f1_loc.ap(), f1T[:])
            if PAIRS:
                nc.gpsimd.collective_compute(
                    "AllGather", mybir.AluOpType.bypass,
                    replica_groups=PAIRS,
                    ins=[f1_loc.ap()], outs=[f1_gath.ap()])
            else:   # single-core build (cost-model runs): fake the gather
                for r in range(HALVES):
                    nc.sync.dma_start(f1_gath.ap()[r], f1_loc.ap())

            with tc.tile_pool(name="poolD", bufs=1) as poolD:
                U2T = [poolD.tile([P, n], F32, tag=f"U2T{o}",
                                  name=f"U2T{o}") for o in range(2)]
                f2T = [poolD.tile([P, nq], F32, tag=f"f2T{o}",
                                  name=f"f2T{o}") for o in range(2)]
                for i in range(nch):
                    sl = bass.ts(i, CH)
                    fch = stg.tile([W1, CH], F32, tag="fch")
                    r = i // (nch // 2)
                    qoff = (i % (nch // 2)) * CH
                    nc.sync.dma_start(fch[:],
                                      f1_gath.ap()[r, :, qoff:qoff + CH])
                    cch = stg.tile([3, CH], F32, tag="cch")
                    nc.sync.dma_start(cch[:], coords_dram.ap()[:, sl])
                    for o in range(2):
                        osl = slice(o * P, o * P + P)
                        mm_chain(U2T[o], sl,
                                 [(W["w2_u_a"][:, osl], fch[:]),
                                  (W["w2_u_b"][:, osl], cch[:])])
                # V2 -> f2T storage
                for i in range(nqch):
                    sl = bass.ts(i, CH)
                    qcch = stg.tile([3, CH], F32, tag="cch")
                    nc.scalar.activation(qcch[:], q5[0:3, sl], Copy,
                                         scale=1.0)
                    for o in range(2):
                        osl = slice(o * P, o * P + P)
                        mm_chain(f2T[o], sl,
                                 [(W["w2_v_a"][:, osl], f1T[:, sl]),
                                  (W["w2_v_b"][:, osl], qcch[:])])

                # ---- loop 2: block2 gather-max -> f2 (in place) ----
                for t in range(nt):
                    tsl = bass.ts(t, P)
                    for o in range(2):
                        gat = gpool.tile([P, P * K], F32, tag="gat")
                        nc.gpsimd.ap_gather(
                            gat[:].rearrange("c (n d) -> c n d", d=1),
                            U2T[o][:].rearrange("c (n d) -> c n d", d=1),
                            wrap_all[:, tsl].bitcast(dt.int16),
                            channels=P, num_elems=n, d=1, num_idxs=P * K)
                        h2 = lpool.tile([P, P], F32, tag="h1")
                        nc.vector.tensor_reduce(
                            h2[:],
                            gat[:].rearrange("c (q s) -> c q s", s=K),
                            axis=AX, op=MAX)
                        nc.vector.tensor_tensor(h2[:], h2[:],
                                                f2T[o][:, tsl], op=ADD)
                        nc.scalar.activation(f2T[o][:, tsl], h2[:], Relu,
                                             bias=W["b2_b"][:, o:o + 1],
                                             scale=1.0)

                # ---- global max pool + glob MLP + b_eff ----
                gmx = pers.tile([P, 2], F32, tag="gmx")
                for o in range(2):
                    nc.vector.tensor_reduce(gmx[:, o:o + 1], f2T[o][:],
                                            axis=AX, op=MAX)
                    nc.sync.dma_start(g_loc.ap()[o * P:o * P + P, :],
                                      gmx[:, o:o + 1])
                if PAIRS:
                    nc.gpsimd.collective_compute(
                        "AllReduce", MAX, replica_groups=PAIRS,
                        ins=[g_loc.ap()], outs=[g_red.ap()])
                else:
                    nc.sync.dma_start(g_red.ap(), g_loc.ap())
                gsb = pers.tile([P, 2], F32, tag="gsb")
                nc.sync.dma_start(
                    gsb[:], g_red.ap().rearrange("(r p) c -> p (r c)", r=2))
                g2 = pers.tile([P, 2], F32, tag="g2")
                beff = pers.tile([P, 2], F32, tag="beff")
                for o in range(2):
                    osl = slice(o * P, o * P + P)
                    mm_chain(g2, slice(o, o + 1),
                             [(W["glob_k0"][:, osl], gsb[:, 0:1]),
                              (W["glob_k1"][:, osl], gsb[:, 1:2])],
                             act=Relu, bias=W["glob_b"][:, o:o + 1],
                             shape=(P, 1))
                for o in range(2):
                    osl = slice(o * P, o * P + P)
                    ps = ppool.tile([P, 1], F32, tag="mm", name="beffps")
                    nc.tensor.matmul(ps[:], W["h1g_k0"][:, osl], g2[:, 0:1],
                                     start=True, stop=False)
                    nc.tensor.matmul(ps[:], W["h1g_k1"][:, osl], g2[:, 1:2],
                                     start=False, stop=True)
                    nc.vector.tensor_scalar(beff[:, o:o + 1], ps[:],
                                            W["h1_b"][:, o:o + 1], None,
                                            op0=ADD)

                # ---- loop 3: head ----
                for t in range(nt):
                    tsl = bass.ts(t, P)
                    hT = [lpool.tile([P, P], F32, tag=f"hT{o}",
                                     name=f"hT{o}") for o in range(2)]
                    for o in range(2):
                        osl = slice(o * P, o * P + P)
                        ps = ppool.tile([P, P], F32, tag="mm", name="hps")
                        nc.tensor.matmul(ps[:], W["h1a_k0"][:, osl],
                                         f2T[0][:, tsl],
                                         start=True, stop=False)
                        nc.tensor.matmul(ps[:], W["h1a_k1"][:, osl],
                                         f2T[1][:, tsl],
                                         start=False, stop=True)
                        nc.scalar.activation(hT[o][:], ps[:], Relu,
                                             bias=beff[:, o:o + 1],
                                             scale=1.0)
                    ps3 = ppool.tile([NUM_CLASSES, P], F32, tag="mm",
                                     name="lps")
                    nc.tensor.matmul(ps3[:], W["h2_k0"][:], hT[0][:],
                                     start=True, stop=False)
                    nc.tensor.matmul(ps3[:], W["h2_k1"][:], hT[1][:],
                                     start=False, stop=True)
                    lg = lpool.tile([NUM_CLASSES, P], F32, tag="lg")
                    nc.vector.tensor_scalar(lg[:], ps3[:],
                                            W["h2_b"][:, 0:1], None,
                                            op0=ADD)
                    sg = lpool.tile([1, P], F32, tag="sg")
                    hagt = lpool.tile([1, P], F32, tag="hagt")
                    nc.sync.dma_start(hagt[:], qxT1.ap()[3:4, tsl])
                    # sig_par: [-sharp, sharp*thresh, scale]
                    nc.scalar.activation(sg[:], hagt[:], Sigmoid,
                                         bias=W["sig_par"][0:1, 1:2],
                                         scale=W["sig_par"][0:1, 0:1])
                    nc.vector.scalar_tensor_tensor(
                        lg[0:1, :], sg[:], W["sig_par"][0:1, 2:3],
                        lg[0:1, :], op0=mybir.AluOpType.mult, op1=ADD)
                    nc.sync.dma_start(out_lg.ap()[:, tsl], lg[:])

    nc.compile()
    return nc


def prep_inputs(x, hmix_a, hmix_b, hmix_c, stem_w, stem_b, b1_w, b1_b,
                b2_w, b2_b, glob_w, glob_b, head1_w, head1_b,
                head2_w, head2_b, thresh, sharp, scale, n=N, ncores=NCORES):
    """Host-side layout prep: per-core input maps (data movement + weight
    repacking only)."""
    f = np.float32
    nq = n // HALVES
    x = np.asarray(x, f)
    one = np.ones((1, n), f)
    m_coords = np.zeros((5, 3), f)
    m_coords[0, 0] = 1.0
    m_coords[1, 1] = 1.0
    m_coords[2, 2] = float(hmix_a)
    m_coords[3, 2] = float(hmix_b)
    m_coords[4, 2] = float(hmix_c)

    b1_w = np.asarray(b1_w, f); b2_w = np.asarray(b2_w, f)
    w1_f, w1_df, w1_dp = b1_w[0:W0], b1_w[W0:2 * W0], b1_w[2 * W0:]
    w2_f, w2_df, w2_dp = b2_w[0:W1], b2_w[W1:2 * W1], b2_w[2 * W1:]
    head1_w = np.asarray(head1_w, f)
    glob_w = np.asarray(glob_w, f); head2_w = np.asarray(head2_w, f)

    com = {
        "m_coords": m_coords,
        "stem_w": np.asarray(stem_w, f),
        "stem_b": np.asarray(stem_b, f).reshape(W0, 1),
        "w1_u_a": np.ascontiguousarray(w1_df),
        "w1_u_b": np.ascontiguousarray(w1_dp),
        "w1_v_a": np.ascontiguousarray(w1_f - w1_df),
        "w1_v_b": np.ascontiguousarray(-w1_dp),
        "b1_b": np.asarray(b1_b, f).reshape(W1, 1),
        "w2_u_a": np.ascontiguousarray(w2_df),
        "w2_u_b": np.ascontiguousarray(w2_dp),
        "w2_v_a": np.ascontiguousarray(w2_f - w2_df),
        "w2_v_b": np.ascontiguousarray(-w2_dp),
        "b2_b": np.ascontiguousarray(np.asarray(b2_b, f).reshape(2, 128).T),
        "glob_k0": np.ascontiguousarray(glob_w[0:128]),
        "glob_k1": np.ascontiguousarray(glob_w[128:256]),
        "glob_b": np.ascontiguousarray(np.asarray(glob_b, f).reshape(2, 128).T),
        "h1a_k0": np.ascontiguousarray(head1_w[0:128]),
        "h1a_k1": np.ascontiguousarray(head1_w[128:256]),
        "h1g_k0": np.ascontiguousarray(head1_w[256:384]),
        "h1g_k1": np.ascontiguousarray(head1_w[384:512]),
        "h1_b": np.ascontiguousarray(np.asarray(head1_b, f).reshape(2, 128).T),
        "h2_k0": np.ascontiguousarray(head2_w[0:128]),
        "h2_k1": np.ascontiguousarray(head2_w[128:256]),
        "h2_b": np.asarray(head2_b, f).reshape(NUM_CLASSES, 1),
        "sig_par": np.array([[-float(sharp), float(sharp) * float(thresh),
                              float(scale)]], f),
    }
    in_maps = []
    for c in range(ncores):
        b, h = c // HALVES, c % HALVES
        xT = np.ascontiguousarray(x[b % B].T[:, :n])
        xT1 = np.concatenate([xT, one], 0)
        qxT1 = np.ascontiguousarray(xT1[:, h * nq:(h + 1) * nq])
        in_maps.append({"xT1": xT1, "qxT1": qxT1, **com})
    return in_maps


_CACHE = {}


def kernel(**inputs):
    from concourse.bass_utils import run_bass_kernel_spmd
    if "nc" not in _CACHE:
        _CACHE["nc"] = build_program()
    nc = _CACHE["nc"]
    in_maps = prep_inputs(**inputs)
    r = run_bass_kernel_spmd(nc, in_maps, list(range(NCORES)))
    nq = N // HALVES
    out = np.zeros((B, N, NUM_CLASSES), np.float32)
    for c in range(NCORES):
        b, h = c // HALVES, c % HALVES
        out[b, h * nq:(h + 1) * nq, :] = r.results[c]["out_lg"].T
    return out



# revision 4
# speedup vs baseline: 1043.4796x; 1043.4796x over previous
"""HeightAwarePointNetTiny on 8 Trainium2 NeuronCores (Bass/Tile).

The reference LocalAggBlock computes, per point i,
    out_i = max_{j in KNN(i)} relu(W [f_i; f_j - f_i; p_j - p_i] + b).
The pre-activation separates into a j-only and an i-only part:
    u_j = W_df f_j + W_dp p_j,   v_i = (W_f - W_df) f_i - W_dp p_i + b
    out_i = relu(v_i + max_{j in KNN(i)} u_j)
so each block is two small matmuls plus a gather-max over the KNN index
lists — no k-wide MLP.  Everything runs channel-major ([C, N]) so matmul
outputs chain without transposes; neighbor gathers use GPSIMD ap_gather
over the free axis.

Sharding: core c owns cloud c//2, query half c%2 (4096 rows).  Cross-core
data: f1 (AllGather over pairs) and the global max pool (AllReduce-max).

KNN top-16 per query row: PE emits score rows s = -dist^2 into PSUM; DVE
max8/max_index/match_replace extract exact top-16 values + indices.
"""
import sys, os
sys.path.insert(0, '/opt/trn_rl_repo')
import numpy as np
from contextlib import ExitStack

import concourse.bass as bass
import concourse.tile as tile
from concourse import bacc, mybir

dt = mybir.dt
F32 = dt.float32

B, N, IN_CH = 4, 8192, 4
K = 16
W0, W1, W2 = 64, 128, 256
NUM_CLASSES = 3
NCORES = 8
P = 128
CH = 512                      # matmul free-dim chunk
HALVES = 2
SEL_CHUNK = int(os.environ.get("SEL_CHUNK", "0"))  # 0 = flat exact top-16


def build_program(n=N, ncores=NCORES, sel_chunk=SEL_CHUNK):
    nq = n // HALVES
    nt = nq // P
    nch = n // CH
    nqch = nq // CH
    nc = bacc.Bacc("TRN2", target_bir_lowering=False, debug=False,
                   num_devices=ncores)

    xT1 = nc.dram_tensor("xT1", [5, n], F32, kind="ExternalInput")
    qxT1 = nc.dram_tensor("qxT1", [5, nq], F32, kind="ExternalInput")
    wm = {}
    for name, shape in [
        ("m_coords", [5, 3]), ("stem_w", [4, W0]), ("stem_b", [W0, 1]),
        ("w1_u_a", [W0, W1]), ("w1_u_b", [3, W1]),
        ("w1_v_a", [W0, W1]), ("w1_v_b", [3, W1]), ("b1_b", [W1, 1]),
        ("w2_u_a", [W1, W2]), ("w2_u_b", [3, W2]),
        ("w2_v_a", [W1, W2]), ("w2_v_b", [3, W2]), ("b2_b", [128, 2]),
        ("glob_k0", [128, W2]), ("glob_k1", [128, W2]), ("glob_b", [128, 2]),
        ("h1a_k0", [128, W2]), ("h1a_k1", [128, W2]),
        ("h1g_k0", [128, W2]), ("h1g_k1", [128, W2]), ("h1_b", [128, 2]),
        ("h2_k0", [128, NUM_CLASSES]), ("h2_k1", [128, NUM_CLASSES]),
        ("h2_b", [NUM_CLASSES, 1]), ("sig_par", [1, 3]),
    ]:
        wm[name] = nc.dram_tensor(name, shape, F32, kind="ExternalInput")

    out_lg = nc.dram_tensor("out_lg", [NUM_CLASSES, nq], F32,
                            kind="ExternalOutput")
    coords_dram = nc.dram_tensor("coords_dram", [3, n], F32)
    f1_loc = nc.dram_tensor("f1_loc", [W1, nq], F32)
    f1_gath = nc.dram_tensor("f1_gath", [HALVES, W1, nq], F32)
    g_loc = nc.dram_tensor("g_loc", [W2, 1], F32)
    g_red = nc.dram_tensor("g_red", [W2, 1], F32)
    PAIRS = [[c, c + 1] for c in range(0, ncores, 2)] if ncores > 1 else []

    Relu = mybir.ActivationFunctionType.Relu
    Copy = mybir.ActivationFunctionType.Copy
    Sigmoid = mybir.ActivationFunctionType.Sigmoid
    Square = mybir.ActivationFunctionType.Square
    AX = mybir.AxisListType.X
    MAX = mybir.AluOpType.max
    ADD = mybir.AluOpType.add

    with tile.TileContext(nc) as tc, ExitStack() as ctx:
        pers = ctx.enter_context(tc.tile_pool(name="pers", bufs=1))
        lpool = ctx.enter_context(tc.tile_pool(name="lp", bufs=2))
        gpool = ctx.enter_context(tc.tile_pool(name="gp", bufs=1))
        stg = ctx.enter_context(tc.tile_pool(name="stg", bufs=2))
        ppool = ctx.enter_context(tc.tile_pool(name="ps", bufs=4, space="PSUM"))

        def mm_chain(dst, dst_sl, parts, act=Copy, bias=0.0, scale=1.0,
                     shape=(P, CH)):
            ps = ppool.tile(list(shape), F32, tag="mm", name="mmps")
            for ix, (lhsT, rhs) in enumerate(parts):
                nc.tensor.matmul(ps[:], lhsT, rhs, start=(ix == 0),
                                 stop=(ix == len(parts) - 1))
            nc.scalar.activation(dst[:, dst_sl], ps[:], act, bias=bias,
                                 scale=scale)

        W = {}
        for name in wm:
            t = pers.tile(list(wm[name].shape), F32, tag=name, name=name)
            nc.sync.dma_start(t[:], wm[name].ap())
            W[name] = t
        wrap_all = pers.tile([P, nt * P], dt.uint16, tag="wrap_all")
        ones3 = pers.tile([3, 1], F32, tag="ones3")
        nc.vector.memset(ones3[:], 1.0)
        cst = pers.tile([1, 2 * CH], F32, tag="cst")
        nc.vector.memset(cst[:, 0:CH], -1.0)
        nc.vector.memset(cst[:, CH:2 * CH], 1.0)

        with tc.tile_pool(name="poolC", bufs=1) as poolC:
            q5 = poolC.tile([5, nq], F32, tag="q5")
            f1T = poolC.tile([W1, nq], F32, tag="f1T")

            with tc.tile_pool(name="poolB", bufs=1) as poolB, \
                 tc.tile_pool(name="spool", bufs=1) as spool:
                rhs5 = poolB.tile([5, n], F32, tag="rhs5")
                U1T = poolB.tile([W1, n], F32, tag="U1T")

                # ---- streamed setup over candidate chunks ----
                for i in range(nch):
                    sl = bass.ts(i, CH)
                    xch = stg.tile([5, CH], F32, tag="xch")
                    nc.sync.dma_start(xch[:], xT1.ap()[:, sl])
                    cch = stg.tile([3, CH], F32, tag="cch")
                    ps = ppool.tile([3, CH], F32, tag="mm", name="csps")
                    nc.tensor.matmul(ps[:], W["m_coords"][:], xch[:],
                                     start=True, stop=True)
                    nc.scalar.activation(cch[:], ps[:], Copy, scale=1.0)
                    nc.scalar.activation(rhs5[0:3, sl], ps[:], Copy,
                                         scale=2.0)
                    nc.sync.dma_start(coords_dram.ap()[:, sl], cch[:])
                    sqs = stg.tile([3, CH], F32, tag="sqs")
                    nc.scalar.activation(sqs[:], cch[:], Square)
                    xxs = stg.tile([1, CH], F32, tag="xxs")
                    mm_chain(xxs, slice(0, CH), [(ones3[:], sqs[:])],
                             scale=-1.0, shape=(1, CH))
                    nc.sync.dma_start(rhs5[4:5, sl], xxs[:])
                    nc.sync.dma_start(rhs5[3:4, sl], cst[0:1, 0:CH])
                    f64 = stg.tile([W0, CH], F32, tag="f64")
                    mm_chain(f64, slice(0, CH),
                             [(W["stem_w"][:], xch[0:4, :])],
                             act=Relu, bias=W["stem_b"][:, 0:1],
                             shape=(W0, CH))
                    mm_chain(U1T, sl, [(W["w1_u_a"][:], f64[:]),
                                       (W["w1_u_b"][:], cch[:])])

                # ---- streamed setup over query chunks (V1 -> f1T) ----
                for i in range(nqch):
                    sl = bass.ts(i, CH)
                    xch = stg.tile([5, CH], F32, tag="xch")
                    nc.sync.dma_start(xch[:], qxT1.ap()[:, sl])
                    ps = ppool.tile([3, CH], F32, tag="mm", name="qcps")
                    nc.tensor.matmul(ps[:], W["m_coords"][:], xch[:],
                                     start=True, stop=True)
                    nc.scalar.activation(q5[0:3, sl], ps[:], Copy, scale=1.0)
                    sqs = stg.tile([3, CH], F32, tag="sqs")
                    nc.scalar.activation(sqs[:], ps[:], Square)
                    xxs = stg.tile([1, CH], F32, tag="xxs")
                    mm_chain(xxs, slice(0, CH), [(ones3[:], sqs[:])],
                             shape=(1, CH))
                    nc.sync.dma_start(q5[3:4, sl], xxs[:])
                    nc.sync.dma_start(q5[4:5, sl], cst[0:1, CH:2 * CH])
                    f64 = stg.tile([W0, CH], F32, tag="f64")
                    mm_chain(f64, slice(0, CH),
                             [(W["stem_w"][:], xch[0:4, :])],
                             act=Relu, bias=W["stem_b"][:, 0:1],
                             shape=(W0, CH))
                    qcch = stg.tile([3, CH], F32, tag="cch")
                    nc.scalar.activation(qcch[:], q5[0:3, sl], Copy,
                                         scale=1.0)
                    mm_chain(f1T, sl, [(W["w1_v_a"][:], f64[:]),
                                       (W["w1_v_b"][:], qcch[:])])

                # ---- loop 1: selection + block1 gather-max ----
                for t in range(nt):
                    tsl = bass.ts(t, P)
                    srow = spool.tile([P, n], F32, tag="srow")
                    for i in range(nch):
                        ps = ppool.tile([P, CH], F32, tag="mm", name="sps")
                        nc.tensor.matmul(ps[:], q5[:, tsl],
                                         rhs5[:, bass.ts(i, CH)],
                                         start=True, stop=True)
                        nc.scalar.activation(srow[:, bass.ts(i, CH)], ps[:],
                                             Copy, scale=1.0)
                    w16 = lpool.tile([P, K], F32, tag="w16")
                    gi = lpool.tile([P, K], dt.uint16, tag="gi")
                    if sel_chunk:
                        nsc = n // sel_chunk
                        cand = lpool.tile([P, 8 * nsc], F32, tag="cand")
                        for j in range(nsc):
                            nc.vector.max(cand[:, bass.ts(j, 8)],
                                          srow[:, bass.ts(j, sel_chunk)])
                        nc.vector.max(w16[:, 0:8], cand[:])
                        nc.vector.match_replace(cand[:], w16[:, 0:8],
                                                cand[:], -3e38)
                        nc.vector.max(w16[:, 8:16], cand[:])
                    else:
                        nc.vector.max(w16[:, 0:8], srow[:])
                        nc.vector.max_index(gi[:, 0:8], w16[:, 0:8],
                                            srow[:])
                        nc.vector.match_replace(srow[:], w16[:, 0:8],
                                                srow[:], -3e38)
                        nc.vector.max(w16[:, 8:16], srow[:])
                    if sel_chunk:
                        nc.vector.max_index(gi[:, 0:8], w16[:, 0:8],
                                            srow[:])
                    nc.vector.max_index(gi[:, 8:16], w16[:, 8:16], srow[:])

                    gip = lpool.tile([P, 32], dt.uint16, tag="gip")
                    nc.vector.memset(gip[:], 0)
                    nc.vector.tensor_copy(gip[:, 0:16], gi[:])
                    giT = lpool.tile([32, P], dt.uint16, tag="giT")
                    for b_ in range(4):
                        nc.vector.transpose(
                            giT[0:32, 32 * b_:32 * b_ + 32],
                            gip[32 * b_:32 * b_ + 32, 0:32])
                    for g in range(8):
                        nc.sync.dma_start(wrap_all[16 * g:16 * g + 16, tsl],
                                          giT[0:16, :])

                    gat = gpool.tile([P, P * K], F32, tag="gat")
                    nc.gpsimd.ap_gather(
                        gat[:].rearrange("c (n d) -> c n d", d=1),
                        U1T[:].rearrange("c (n d) -> c n d", d=1),
                        wrap_all[:, tsl].bitcast(dt.int16),
                        channels=P, num_elems=n, d=1, num_idxs=P * K)
                    h1 = lpool.tile([P, P], F32, tag="h1")
                    nc.vector.tensor_reduce(
                        h1[:], gat[:].rearrange("c (q s) -> c q s", s=K),
                        axis=AX, op=MAX)
                    nc.vector.tensor_tensor(h1[:], h1[:], f1T[:, tsl],
                                            op=ADD)
                    nc.scalar.activation(f1T[:, tsl], h1[:], Relu,
                                         bias=W["b1_b"][:, 0:1], scale=1.0)

            # ---- exchange f1 halves within the pair ----
            nc.sync.dma_start(f1_loc.ap(), f1T[:])
            if PAIRS:
                nc.gpsimd.collective_compute(
                    "AllGather", mybir.AluOpType.bypass,
                    replica_groups=PAIRS,
                    ins=[f1_loc.ap()], outs=[f1_gath.ap()])
            else:   # single-core build (cost-model runs): fake the gather
                for r in range(HALVES):
                    nc.sync.dma_start(f1_gath.ap()[r], f1_loc.ap())

            with tc.tile_pool(name="poolD", bufs=1) as poolD:
                U2T = [poolD.tile([P, n], F32, tag=f"U2T{o}",
                                  name=f"U2T{o}") for o in range(2)]
                f2T = [poolD.tile([P, nq], F32, tag=f"f2T{o}",
                                  name=f"f2T{o}") for o in range(2)]
                for i in range(nch):
                    sl = bass.ts(i, CH)
                    fch = stg.tile([W1, CH], F32, tag="fch")
                    r = i // (nch // 2)
                    qoff = (i % (nch // 2)) * CH
                    nc.sync.dma_start(fch[:],
                                      f1_gath.ap()[r, :, qoff:qoff + CH])
                    cch = stg.tile([3, CH], F32, tag="cch")
                    nc.sync.dma_start(cch[:], coords_dram.ap()[:, sl])
                    for o in range(2):
                        osl = slice(o * P, o * P + P)
                        mm_chain(U2T[o], sl,
                                 [(W["w2_u_a"][:, osl], fch[:]),
                                  (W["w2_u_b"][:, osl], cch[:])])
                # V2 -> f2T storage
                for i in range(nqch):
                    sl = bass.ts(i, CH)
                    qcch = stg.tile([3, CH], F32, tag="cch")
                    nc.scalar.activation(qcch[:], q5[0:3, sl], Copy,
                                         scale=1.0)
                    for o in range(2):
                        osl = slice(o * P, o * P + P)
                        mm_chain(f2T[o], sl,
                                 [(W["w2_v_a"][:, osl], f1T[:, sl]),
                                  (W["w2_v_b"][:, osl], qcch[:])])

                # ---- loop 2: block2 gather-max -> f2 (in place) ----
                for t in range(nt):
                    tsl = bass.ts(t, P)
                    for o in range(2):
                        gat = gpool.tile([P, P * K], F32, tag="gat")
                        nc.gpsimd.ap_gather(
                            gat[:].rearrange("c (n d) -> c n d", d=1),
                            U2T[o][:].rearrange("c (n d) -> c n d", d=1),
                            wrap_all[:, tsl].bitcast(dt.int16),
                            channels=P, num_elems=n, d=1, num_idxs=P * K)
                        h2 = lpool.tile([P, P], F32, tag="h1")
                        nc.vector.tensor_reduce(
                            h2[:],
                            gat[:].rearrange("c (q s) -> c q s", s=K),
                            axis=AX, op=MAX)
                        nc.vector.tensor_tensor(h2[:], h2[:],
                                                f2T[o][:, tsl], op=ADD)
                        nc.scalar.activation(f2T[o][:, tsl], h2[:], Relu,
                                             bias=W["b2_b"][:, o:o + 1],
                                             scale=1.0)

                # ---- global max pool + glob MLP + b_eff ----
                gmx = pers.tile([P, 2], F32, tag="gmx")
                for o in range(2):
                    nc.vector.tensor_reduce(gmx[:, o:o + 1], f2T[o][:],
                                            axis=AX, op=MAX)
                    nc.sync.dma_start(g_loc.ap()[o * P:o * P + P, :],
                                      gmx[:, o:o + 1])
                if PAIRS:
                    nc.gpsimd.collective_compute(
                        "AllReduce", MAX, replica_groups=PAIRS,
                        ins=[g_loc.ap()], outs=[g_red.ap()])
                else:
                    nc.sync.dma_start(g_red.ap(), g_loc.ap())
                gsb = pers.tile([P, 2], F32, tag="gsb")
                nc.sync.dma_start(
                    gsb[:], g_red.ap().rearrange("(r p) c -> p (r c)", r=2))
                g2 = pers.tile([P, 2], F32, tag="g2")
                beff = pers.tile([P, 2], F32, tag="beff")
                for o in range(2):
                    osl = slice(o * P, o * P + P)
                    mm_chain(g2, slice(o, o + 1),
                             [(W["glob_k0"][:, osl], gsb[:, 0:1]),
                              (W["glob_k1"][:, osl], gsb[:, 1:2])],
                             act=Relu, bias=W["glob_b"][:, o:o + 1],
                             shape=(P, 1))
                for o in range(2):
                    osl = slice(o * P, o * P + P)
                    ps = ppool.tile([P, 1], F32, tag="mm", name="beffps")
                    nc.tensor.matmul(ps[:], W["h1g_k0"][:, osl], g2[:, 0:1],
                                     start=True, stop=False)
                    nc.tensor.matmul(ps[:], W["h1g_k1"][:, osl], g2[:, 1:2],
                                     start=False, stop=True)
                    nc.vector.tensor_scalar(beff[:, o:o + 1], ps[:],
                                            W["h1_b"][:, o:o + 1], None,
                                            op0=ADD)

                # ---- loop 3: head ----
                for t in range(nt):
                    tsl = bass.ts(t, P)
                    hT = [lpool.tile([P, P], F32, tag=f"hT{o}",
                                     name=f"hT{o}") for o in range(2)]
                    for o in range(2):
                        osl = slice(o * P, o * P + P)
                        ps = ppool.tile([P, P], F32, tag="mm", name="hps")
                        nc.tensor.matmul(ps[:], W["h1a_k0"][:, osl],
                                         f2T[0][:, tsl],
                                         start=True, stop=False)
                        nc.tensor.matmul(ps[:], W["h1a_k1"][:, osl],
                                         f2T[1][:, tsl],
                                         start=False, stop=True)
                        nc.scalar.activation(hT[o][:], ps[:], Relu,
                                             bias=beff[:, o:o + 1],
                                             scale=1.0)
                    ps3 = ppool.tile([NUM_CLASSES, P], F32, tag="mm",
                                     name="lps")
                    nc.tensor.matmul(ps3[:], W["h2_k0"][:], hT[0][:],
                                     start=True, stop=False)
                    nc.tensor.matmul(ps3[:], W["h2_k1"][:], hT[1][:],
                                     start=False, stop=True)
                    lg = lpool.tile([NUM_CLASSES, P], F32, tag="lg")
                    nc.vector.tensor_scalar(lg[:], ps3[:],
                                            W["h2_b"][:, 0:1], None,
                                            op0=ADD)
                    sg = lpool.tile([1, P], F32, tag="sg")
                    hagt = lpool.tile([1, P], F32, tag="hagt")
                    nc.sync.dma_start(hagt[:], qxT1.ap()[3:4, tsl])
                    # sig_par: [-sharp, sharp*thresh, scale]
                    nc.scalar.activation(sg[:], hagt[:], Sigmoid,
                                         bias=W["sig_par"][0:1, 1:2],
                                         scale=W["sig_par"][0:1, 0:1])
                    nc.vector.scalar_tensor_tensor(
                        lg[0:1, :], sg[:], W["sig_par"][0:1, 2:3],
                        lg[0:1, :], op0=mybir.AluOpType.mult, op1=ADD)
                    nc.sync.dma_start(out_lg.ap()[:, tsl], lg[:])

    nc.compile()
    return nc


def prep_inputs(x, hmix_a, hmix_b, hmix_c, stem_w, stem_b, b1_w, b1_b,
                b2_w, b2_b, glob_w, glob_b, head1_w, head1_b,
                head2_w, head2_b, thresh, sharp, scale, n=N, ncores=NCORES):
    """Host-side layout prep: per-core input maps (data movement + weight
    repacking only)."""
    f = np.float32
    nq = n // HALVES
    x = np.asarray(x, f)
    one = np.ones((1, n), f)
    m_coords = np.zeros((5, 3), f)
    m_coords[0, 0] = 1.0
    m_coords[1, 1] = 1.0
    m_coords[2, 2] = float(hmix_a)
    m_coords[3, 2] = float(hmix_b)
    m_coords[4, 2] = float(hmix_c)

    b1_w = np.asarray(b1_w, f); b2_w = np.asarray(b2_w, f)
    w1_f, w1_df, w1_dp = b1_w[0:W0], b1_w[W0:2 * W0], b1_w[2 * W0:]
    w2_f, w2_df, w2_dp = b2_w[0:W1], b2_w[W1:2 * W1], b2_w[2 * W1:]
    head1_w = np.asarray(head1_w, f)
    glob_w = np.asarray(glob_w, f); head2_w = np.asarray(head2_w, f)

    com = {
        "m_coords": m_coords,
        "stem_w": np.asarray(stem_w, f),
        "stem_b": np.asarray(stem_b, f).reshape(W0, 1),
        "w1_u_a": np.ascontiguousarray(w1_df),
        "w1_u_b": np.ascontiguousarray(w1_dp),
        "w1_v_a": np.ascontiguousarray(w1_f - w1_df),
        "w1_v_b": np.ascontiguousarray(-w1_dp),
        "b1_b": np.asarray(b1_b, f).reshape(W1, 1),
        "w2_u_a": np.ascontiguousarray(w2_df),
        "w2_u_b": np.ascontiguousarray(w2_dp),
        "w2_v_a": np.ascontiguousarray(w2_f - w2_df),
        "w2_v_b": np.ascontiguousarray(-w2_dp),
        "b2_b": np.ascontiguousarray(np.asarray(b2_b, f).reshape(2, 128).T),
        "glob_k0": np.ascontiguousarray(glob_w[0:128]),
        "glob_k1": np.ascontiguousarray(glob_w[128:256]),
        "glob_b": np.ascontiguousarray(np.asarray(glob_b, f).reshape(2, 128).T),
        "h1a_k0": np.ascontiguousarray(head1_w[0:128]),
        "h1a_k1": np.ascontiguousarray(head1_w[128:256]),
        "h1g_k0": np.ascontiguousarray(head1_w[256:384]),
        "h1g_k1": np.ascontiguousarray(head1_w[384:512]),
        "h1_b": np.ascontiguousarray(np.asarray(head1_b, f).reshape(2, 128).T),
        "h2_k0": np.ascontiguousarray(head2_w[0:128]),
        "h2_k1": np.ascontiguousarray(head2_w[128:256]),
        "h2_b": np.asarray(head2_b, f).reshape(NUM_CLASSES, 1),
        "sig_par": np.array([[-float(sharp), float(sharp) * float(thresh),
                              float(scale)]], f),
    }
    in_maps = []
    for c in range(ncores):
        b, h = c // HALVES, c % HALVES
        xT = np.ascontiguousarray(x[b % B].T[:, :n])
        xT1 = np.concatenate([xT, one], 0)
        qxT1 = np.ascontiguousarray(xT1[:, h * nq:(h + 1) * nq])
        in_maps.append({"xT1": xT1, "qxT1": qxT1, **com})
    return in_maps


_CACHE = {}


def kernel(**inputs):
    from concourse.bass_utils import run_bass_kernel_spmd
    if "nc" not in _CACHE:
        _CACHE["nc"] = build_program()
    nc = _CACHE["nc"]
    in_maps = prep_inputs(**inputs)
    r = run_bass_kernel_spmd(nc, in_maps, list(range(NCORES)))
    nq = N // HALVES
    out = np.zeros((B, N, NUM_CLASSES), np.float32)
    for c in range(NCORES):
        b, h = c // HALVES, c % HALVES
        out[b, h * nq:(h + 1) * nq, :] = r.results[c]["out_lg"].T
    return out



# revision 6
# speedup vs baseline: 1632.9167x; 1.5649x over previous
"""HeightAwarePointNetTiny on 8 Trainium2 NeuronCores (Bass/Tile).

The reference LocalAggBlock computes, per point i,
    out_i = max_{j in KNN(i)} relu(W [f_i; f_j - f_i; p_j - p_i] + b).
The pre-activation separates into a j-only and an i-only part:
    u_j = W_df f_j + W_dp p_j,   v_i = (W_f - W_df) f_i - W_dp p_i + b
    out_i = relu(v_i + max_{j in KNN(i)} u_j)
so each block is two small matmuls plus a gather-max over the KNN index
lists — no k-wide MLP.  Everything runs channel-major ([C, N]) so matmul
outputs chain without transposes; neighbor gathers use GPSIMD ap_gather
over the free axis.

Sharding: core c owns cloud c//2, query half c%2 (4096 rows).  Cross-core
data: f1 (AllGather over pairs) and the global max pool (AllReduce-max).

KNN top-16 per query row: PE emits score rows s = -dist^2 into PSUM; DVE
max8/max_index/match_replace extract exact top-16 values + indices.
"""
import sys, os
sys.path.insert(0, '/opt/trn_rl_repo')
import numpy as np
from contextlib import ExitStack

import concourse.bass as bass
import concourse.tile as tile
from concourse import bacc, mybir

dt = mybir.dt
F32 = dt.float32
BF16 = dt.bfloat16

B, N, IN_CH = 4, 8192, 4
K = 16
W0, W1, W2 = 64, 128, 256
NUM_CLASSES = 3
NCORES = 8
P = 128
CH = 512                      # matmul free-dim chunk
HALVES = 2
SEL_CHUNK = int(os.environ.get("SEL_CHUNK", "0"))  # 0 = flat exact top-16


def build_program(n=N, ncores=NCORES, sel_chunk=SEL_CHUNK):
    nq = n // HALVES
    nt = nq // P
    nch = n // CH
    nqch = nq // CH
    nc = bacc.Bacc("TRN2", target_bir_lowering=False, debug=False,
                   num_devices=ncores)

    xT1 = nc.dram_tensor("xT1", [5, n], F32, kind="ExternalInput")
    qxT1 = nc.dram_tensor("qxT1", [5, nq], F32, kind="ExternalInput")
    wm = {}
    for name, shape in [
        ("m_coords", [5, 3]), ("stem_w", [4, W0]), ("stem_b", [W0, 1]),
        ("w1_u_a", [W0, W1]), ("w1_u_b", [3, W1]),
        ("w1_v_a", [W0, W1]), ("w1_v_b", [3, W1]), ("b1_b", [W1, 1]),
        ("w2_u_a", [W1, W2]), ("w2_u_b", [3, W2]),
        ("w2_v_a", [W1, W2]), ("w2_v_b", [3, W2]), ("b2_b", [128, 2]),
        ("glob_k0", [128, W2]), ("glob_k1", [128, W2]), ("glob_b", [128, 2]),
        ("h1a_k0", [128, W2]), ("h1a_k1", [128, W2]),
        ("h1g_k0", [128, W2]), ("h1g_k1", [128, W2]), ("h1_b", [128, 2]),
        ("h2_k0", [128, NUM_CLASSES]), ("h2_k1", [128, NUM_CLASSES]),
        ("h2_b", [NUM_CLASSES, 1]), ("sig_par", [1, 3]),
        ("choff", [P, P]),
    ]:
        wm[name] = nc.dram_tensor(name, shape, F32, kind="ExternalInput")

    out_lg = nc.dram_tensor("out_lg", [NUM_CLASSES, nq], F32,
                            kind="ExternalOutput")
    coords_dram = nc.dram_tensor("coords_dram", [3, n], F32)
    f1_loc = nc.dram_tensor("f1_loc", [W1, nq], F32)
    f1_gath = nc.dram_tensor("f1_gath", [HALVES, W1, nq], F32)
    g_loc = nc.dram_tensor("g_loc", [W2, 1], F32)
    g_red = nc.dram_tensor("g_red", [W2, 1], F32)
    PAIRS = [[c, c + 1] for c in range(0, ncores, 2)] if ncores > 1 else []

    Relu = mybir.ActivationFunctionType.Relu
    Copy = mybir.ActivationFunctionType.Copy
    Sigmoid = mybir.ActivationFunctionType.Sigmoid
    Square = mybir.ActivationFunctionType.Square
    AX = mybir.AxisListType.X
    MAX = mybir.AluOpType.max
    ADD = mybir.AluOpType.add

    with tile.TileContext(nc) as tc, ExitStack() as ctx:
        pers = ctx.enter_context(tc.tile_pool(name="pers", bufs=1))
        lpool = ctx.enter_context(tc.tile_pool(name="lp", bufs=2))
        gpool = ctx.enter_context(tc.tile_pool(name="gp", bufs=2))
        stg = ctx.enter_context(tc.tile_pool(name="stg", bufs=2))
        ppool = ctx.enter_context(tc.tile_pool(name="ps", bufs=4, space="PSUM"))

        def mm_chain(dst, dst_sl, parts, act=Copy, bias=0.0, scale=1.0,
                     shape=(P, CH)):
            ps = ppool.tile(list(shape), F32, tag="mm", name="mmps")
            for ix, (lhsT, rhs) in enumerate(parts):
                nc.tensor.matmul(ps[:], lhsT, rhs, start=(ix == 0),
                                 stop=(ix == len(parts) - 1))
            nc.scalar.activation(dst[:, dst_sl], ps[:], act, bias=bias,
                                 scale=scale)

        W = {}
        for name in wm:
            t = pers.tile(list(wm[name].shape), F32, tag=name, name=name)
            nc.sync.dma_start(t[:], wm[name].ap())
            W[name] = t
        wrap_all = pers.tile([P, nt * P], dt.uint16, tag="wrap_all")
        choffu = pers.tile([P, P], dt.uint16, tag="choffu")
        nc.vector.tensor_copy(choffu[:], W["choff"][:])
        MARK, MARKV, BIGNEG = float(2 ** 20), 1e30, -3e38
        MULT = mybir.AluOpType.mult
        SUB = mybir.AluOpType.subtract
        MIN = mybir.AluOpType.min
        ISEQ = mybir.AluOpType.is_equal
        ones3 = pers.tile([3, 1], F32, tag="ones3")
        nc.vector.memset(ones3[:], 1.0)
        cst = pers.tile([1, 2 * CH], F32, tag="cst")
        nc.vector.memset(cst[:, 0:CH], -1.0)
        nc.vector.memset(cst[:, CH:2 * CH], 1.0)

        with tc.tile_pool(name="poolC", bufs=1) as poolC:
            q5 = poolC.tile([5, nq], F32, tag="q5")
            f1T = poolC.tile([W1, nq], F32, tag="f1T")

            with tc.tile_pool(name="poolB", bufs=1) as poolB, \
                 tc.tile_pool(name="spool", bufs=1) as spool:
                rhs5 = poolB.tile([5, n], F32, tag="rhs5")
                U1T = poolB.tile([W1, n], F32, tag="U1T")

                # ---- streamed setup over candidate chunks ----
                for i in range(nch):
                    sl = bass.ts(i, CH)
                    xch = stg.tile([5, CH], F32, tag="xch")
                    nc.sync.dma_start(xch[:], xT1.ap()[:, sl])
                    cch = stg.tile([3, CH], F32, tag="cch")
                    ps = ppool.tile([3, CH], F32, tag="mm", name="csps")
                    nc.tensor.matmul(ps[:], W["m_coords"][:], xch[:],
                                     start=True, stop=True)
                    nc.scalar.activation(cch[:], ps[:], Copy, scale=1.0)
                    nc.scalar.activation(rhs5[0:3, sl], ps[:], Copy,
                                         scale=2.0)
                    nc.sync.dma_start(coords_dram.ap()[:, sl], cch[:])
                    sqs = stg.tile([3, CH], F32, tag="sqs")
                    nc.scalar.activation(sqs[:], cch[:], Square)
                    xxs = stg.tile([1, CH], F32, tag="xxs")
                    mm_chain(xxs, slice(0, CH), [(ones3[:], sqs[:])],
                             scale=-1.0, shape=(1, CH))
                    nc.sync.dma_start(rhs5[4:5, sl], xxs[:])
                    nc.sync.dma_start(rhs5[3:4, sl], cst[0:1, 0:CH])
                    f64 = stg.tile([W0, CH], F32, tag="f64")
                    mm_chain(f64, slice(0, CH),
                             [(W["stem_w"][:], xch[0:4, :])],
                             act=Relu, bias=W["stem_b"][:, 0:1],
                             shape=(W0, CH))
                    mm_chain(U1T, sl, [(W["w1_u_a"][:], f64[:]),
                                       (W["w1_u_b"][:], cch[:])])

                # ---- streamed setup over query chunks (V1 -> f1T) ----
                for i in range(nqch):
                    sl = bass.ts(i, CH)
                    xch = stg.tile([5, CH], F32, tag="xch")
                    nc.sync.dma_start(xch[:], qxT1.ap()[:, sl])
                    ps = ppool.tile([3, CH], F32, tag="mm", name="qcps")
                    nc.tensor.matmul(ps[:], W["m_coords"][:], xch[:],
                                     start=True, stop=True)
                    nc.scalar.activation(q5[0:3, sl], ps[:], Copy, scale=1.0)
                    sqs = stg.tile([3, CH], F32, tag="sqs")
                    nc.scalar.activation(sqs[:], ps[:], Square)
                    xxs = stg.tile([1, CH], F32, tag="xxs")
                    mm_chain(xxs, slice(0, CH), [(ones3[:], sqs[:])],
                             shape=(1, CH))
                    nc.sync.dma_start(q5[3:4, sl], xxs[:])
                    nc.sync.dma_start(q5[4:5, sl], cst[0:1, CH:2 * CH])
                    f64 = stg.tile([W0, CH], F32, tag="f64")
                    mm_chain(f64, slice(0, CH),
                             [(W["stem_w"][:], xch[0:4, :])],
                             act=Relu, bias=W["stem_b"][:, 0:1],
                             shape=(W0, CH))
                    qcch = stg.tile([3, CH], F32, tag="cch")
                    nc.scalar.activation(qcch[:], q5[0:3, sl], Copy,
                                         scale=1.0)
                    mm_chain(f1T, sl, [(W["w1_v_a"][:], f64[:]),
                                       (W["w1_v_b"][:], qcch[:])])

                # ---- loop 1: chunked 2-pass selection + block1 ----
                pend1 = []
                for t in range(nt):
                    tsl = bass.ts(t, P)
                    cand = lpool.tile([P, P], F32, tag="cand")
                    lidx = lpool.tile([P, P], dt.uint16, tag="lidx")
                    for c in range(nch):
                        ps = ppool.tile([P, CH], F32, tag="mm", name="sps")
                        nc.tensor.matmul(ps[:], q5[:, tsl],
                                         rhs5[:, bass.ts(c, CH)],
                                         start=True, stop=True)
                        srow = spool.tile([P, CH], F32, tag="srow")
                        nc.scalar.activation(srow[:], ps[:], Copy, scale=1.0)
                        c8 = slice(8 * c, 8 * c + 8)
                        nc.vector.max(cand[:, c8], srow[:])
                        nc.vector.max_index(lidx[:, c8], cand[:, c8],
                                            srow[:])
                    # global idx = local + 512*chunk; keep an f32 copy
                    nc.vector.tensor_tensor(lidx[:], lidx[:], choffu[:],
                                            op=ADD)
                    lidxf = lpool.tile([P, P], F32, tag="lidxf")
                    nc.vector.tensor_copy(lidxf[:], lidx[:])
                    # top-16 merge; winners marked in place with -MARKV
                    w16 = lpool.tile([P, K], F32, tag="w16")
                    nc.vector.max(w16[:, 0:8], cand[:])
                    nc.vector.match_replace(cand[:], w16[:, 0:8], cand[:],
                                            -MARKV)
                    nc.vector.max(w16[:, 8:16], cand[:])
                    nc.vector.match_replace(cand[:], w16[:, 8:16], cand[:],
                                            -MARKV)
                    mk = lpool.tile([P, P], F32, tag="mk")
                    nc.vector.tensor_scalar(mk[:], cand[:], -MARKV, None,
                                            op0=ISEQ)
                    pk = lpool.tile([P, P], F32, tag="pk")
                    nc.vector.scalar_tensor_tensor(pk[:], mk[:], MARK,
                                                   lidxf[:], op0=MULT,
                                                   op1=SUB)
                    gp16 = lpool.tile([P, K], F32, tag="gp16")
                    nc.vector.max(gp16[:, 0:8], pk[:])
                    nc.vector.match_replace(pk[:], gp16[:, 0:8], pk[:],
                                            BIGNEG)
                    nc.vector.max(gp16[:, 8:16], pk[:])
                    gf = lpool.tile([P, K], F32, tag="gf")
                    nc.vector.tensor_scalar(gf[:], gp16[:], -1.0, MARK,
                                            op0=MULT, op1=ADD)
                    gip = lpool.tile([P, 32], dt.uint16, tag="gip")
                    nc.vector.memset(gip[:, 16:32], 0)
                    nc.vector.tensor_scalar(gip[:, 0:16], gf[:],
                                            float(n - 1), None, op0=MIN)
                    giT = lpool.tile([32, P], dt.uint16, tag="giT")
                    for b_ in range(4):
                        nc.vector.transpose(
                            giT[0:32, 32 * b_:32 * b_ + 32],
                            gip[32 * b_:32 * b_ + 32, 0:32])
                    for g in range(8):
                        nc.sync.dma_start(wrap_all[16 * g:16 * g + 16, tsl],
                                          giT[0:16, :])

                    gat = gpool.tile([P, P * K], F32, tag="gat")
                    nc.gpsimd.ap_gather(
                        gat[:].rearrange("c (n d) -> c n d", d=1),
                        U1T[:].rearrange("c (n d) -> c n d", d=1),
                        wrap_all[:, tsl].bitcast(dt.int16),
                        channels=P, num_elems=n, d=1, num_idxs=P * K)
                    pend1.append((tsl, gat))
                    if len(pend1) > 1 or t == nt - 1:
                        for psl, pgat in pend1[:None if t == nt - 1 else -1]:
                            h1 = lpool.tile([P, P], F32, tag="h1")
                            nc.vector.tensor_reduce(
                                h1[:],
                                pgat[:].rearrange("c (q s) -> c q s", s=K),
                                axis=AX, op=MAX)
                            nc.vector.tensor_tensor(h1[:], h1[:],
                                                    f1T[:, psl], op=ADD)
                            nc.scalar.activation(f1T[:, psl], h1[:], Relu,
                                                 bias=W["b1_b"][:, 0:1],
                                                 scale=1.0)
                        pend1 = pend1[-1:] if t != nt - 1 else []

            # ---- exchange f1 halves within the pair ----
            nc.sync.dma_start(f1_loc.ap(), f1T[:])
            if PAIRS:
                nc.gpsimd.collective_compute(
                    "AllGather", mybir.AluOpType.bypass,
                    replica_groups=PAIRS,
                    ins=[f1_loc.ap()], outs=[f1_gath.ap()])
            else:   # single-core build (cost-model runs): fake the gather
                for r in range(HALVES):
                    nc.sync.dma_start(f1_gath.ap()[r], f1_loc.ap())

            with tc.tile_pool(name="poolD", bufs=1) as poolD:
                # both 128-channel halves of u2, bf16, interleaved per point
                # so one ap_gather (f32 view) fetches them together
                U2P = poolD.tile([P, 2 * n], BF16, tag="U2P")
                f2T = [poolD.tile([P, nq], F32, tag=f"f2T{o}",
                                  name=f"f2T{o}") for o in range(2)]
                U2Pv = U2P[:].rearrange("c (x h) -> c h x", h=2)
                for i in range(nch):
                    sl = bass.ts(i, CH)
                    fch = stg.tile([W1, CH], F32, tag="fch")
                    r = i // (nch // 2)
                    qoff = (i % (nch // 2)) * CH
                    nc.sync.dma_start(fch[:],
                                      f1_gath.ap()[r, :, qoff:qoff + CH])
                    cch = stg.tile([3, CH], F32, tag="cch")
                    nc.sync.dma_start(cch[:], coords_dram.ap()[:, sl])
                    for o in range(2):
                        osl = slice(o * P, o * P + P)
                        ps = ppool.tile([P, CH], F32, tag="mm", name="u2ps")
                        nc.tensor.matmul(ps[:], W["w2_u_a"][:, osl], fch[:],
                                         start=True, stop=False)
                        nc.tensor.matmul(ps[:], W["w2_u_b"][:, osl], cch[:],
                                         start=False, stop=True)
                        nc.scalar.activation(
                            U2Pv[:, o:o + 1, sl],
                            ps[:].rearrange("c (o x) -> c o x", o=1), Copy)
                # V2 -> f2T storage
                for i in range(nqch):
                    sl = bass.ts(i, CH)
                    qcch = stg.tile([3, CH], F32, tag="cch")
                    nc.scalar.activation(qcch[:], q5[0:3, sl], Copy,
                                         scale=1.0)
                    for o in range(2):
                        osl = slice(o * P, o * P + P)
                        mm_chain(f2T[o], sl,
                                 [(W["w2_v_a"][:, osl], f1T[:, sl]),
                                  (W["w2_v_b"][:, osl], qcch[:])])

                # ---- loop 2: block2 gather-max -> f2 (in place) ----
                pend2 = []
                for t in range(nt):
                    tsl = bass.ts(t, P)
                    gat2 = gpool.tile([P, P * K], F32, tag="gat2")
                    nc.gpsimd.ap_gather(
                        gat2[:].rearrange("c (n d) -> c n d", d=1),
                        U2P[:].bitcast(F32).rearrange("c (n d) -> c n d",
                                                      d=1),
                        wrap_all[:, tsl].bitcast(dt.int16),
                        channels=P, num_elems=n, d=1, num_idxs=P * K)
                    pend2.append((tsl, gat2))
                    if len(pend2) > 1 or t == nt - 1:
                        for psl, pgat in pend2[:None if t == nt - 1 else -1]:
                            gb = pgat[:].bitcast(BF16).rearrange(
                                "c (q s h) -> c q h s", s=K, h=2)
                            for o in range(2):
                                h2 = lpool.tile([P, P], F32, tag="h1")
                                nc.vector.tensor_reduce(
                                    h2[:].rearrange("c (q o) -> c q o", o=1),
                                    gb[:, :, o:o + 1, :], axis=AX, op=MAX)
                                nc.vector.tensor_tensor(h2[:], h2[:],
                                                        f2T[o][:, psl],
                                                        op=ADD)
                                nc.scalar.activation(f2T[o][:, psl], h2[:],
                                                     Relu,
                                                     bias=W["b2_b"][:,
                                                                    o:o + 1],
                                                     scale=1.0)
                        pend2 = pend2[-1:] if t != nt - 1 else []

                # ---- global max pool + glob MLP + b_eff ----
                gmx = pers.tile([P, 2], F32, tag="gmx")
                for o in range(2):
                    nc.vector.tensor_reduce(gmx[:, o:o + 1], f2T[o][:],
                                            axis=AX, op=MAX)
                    nc.sync.dma_start(g_loc.ap()[o * P:o * P + P, :],
                                      gmx[:, o:o + 1])
                if PAIRS:
                    nc.gpsimd.collective_compute(
                        "AllReduce", MAX, replica_groups=PAIRS,
                        ins=[g_loc.ap()], outs=[g_red.ap()])
                else:
                    nc.sync.dma_start(g_red.ap(), g_loc.ap())
                gsb = pers.tile([P, 2], F32, tag="gsb")
                nc.sync.dma_start(
                    gsb[:], g_red.ap().rearrange("(r p) c -> p (r c)", r=2))
                g2 = pers.tile([P, 2], F32, tag="g2")
                beff = pers.tile([P, 2], F32, tag="beff")
                for o in range(2):
                    osl = slice(o * P, o * P + P)
                    mm_chain(g2, slice(o, o + 1),
                             [(W["glob_k0"][:, osl], gsb[:, 0:1]),
                              (W["glob_k1"][:, osl], gsb[:, 1:2])],
                             act=Relu, bias=W["glob_b"][:, o:o + 1],
                             shape=(P, 1))
                for o in range(2):
                    osl = slice(o * P, o * P + P)
                    ps = ppool.tile([P, 1], F32, tag="mm", name="beffps")
                    nc.tensor.matmul(ps[:], W["h1g_k0"][:, osl], g2[:, 0:1],
                                     start=True, stop=False)
                    nc.tensor.matmul(ps[:], W["h1g_k1"][:, osl], g2[:, 1:2],
                                     start=False, stop=True)
                    nc.vector.tensor_scalar(beff[:, o:o + 1], ps[:],
                                            W["h1_b"][:, o:o + 1], None,
                                            op0=ADD)

                # ---- loop 3: head ----
                for t in range(nt):
                    tsl = bass.ts(t, P)
                    hT = [lpool.tile([P, P], F32, tag=f"hT{o}",
                                     name=f"hT{o}") for o in range(2)]
                    for o in range(2):
                        osl = slice(o * P, o * P + P)
                        ps = ppool.tile([P, P], F32, tag="mm", name="hps")
                        nc.tensor.matmul(ps[:], W["h1a_k0"][:, osl],
                                         f2T[0][:, tsl],
                                         start=True, stop=False)
                        nc.tensor.matmul(ps[:], W["h1a_k1"][:, osl],
                                         f2T[1][:, tsl],
                                         start=False, stop=True)
                        nc.scalar.activation(hT[o][:], ps[:], Relu,
                                             bias=beff[:, o:o + 1],
                                             scale=1.0)
                    ps3 = ppool.tile([NUM_CLASSES, P], F32, tag="mm",
                                     name="lps")
                    nc.tensor.matmul(ps3[:], W["h2_k0"][:], hT[0][:],
                                     start=True, stop=False)
                    nc.tensor.matmul(ps3[:], W["h2_k1"][:], hT[1][:],
                                     start=False, stop=True)
                    lg = lpool.tile([NUM_CLASSES, P], F32, tag="lg")
                    nc.vector.tensor_scalar(lg[:], ps3[:],
                                            W["h2_b"][:, 0:1], None,
                                            op0=ADD)
                    sg = lpool.tile([1, P], F32, tag="sg")
                    hagt = lpool.tile([1, P], F32, tag="hagt")
                    nc.sync.dma_start(hagt[:], qxT1.ap()[3:4, tsl])
                    # sig_par: [-sharp, sharp*thresh, scale]
                    nc.scalar.activation(sg[:], hagt[:], Sigmoid,
                                         bias=W["sig_par"][0:1, 1:2],
                                         scale=W["sig_par"][0:1, 0:1])
                    nc.vector.scalar_tensor_tensor(
                        lg[0:1, :], sg[:], W["sig_par"][0:1, 2:3],
                        lg[0:1, :], op0=mybir.AluOpType.mult, op1=ADD)
                    nc.sync.dma_start(out_lg.ap()[:, tsl], lg[:])

    nc.compile()
    return nc


def prep_inputs(x, hmix_a, hmix_b, hmix_c, stem_w, stem_b, b1_w, b1_b,
                b2_w, b2_b, glob_w, glob_b, head1_w, head1_b,
                head2_w, head2_b, thresh, sharp, scale, n=N, ncores=NCORES):
    """Host-side layout prep: per-core input maps (data movement + weight
    repacking only)."""
    f = np.float32
    nq = n // HALVES
    x = np.asarray(x, f)
    one = np.ones((1, n), f)
    m_coords = np.zeros((5, 3), f)
    m_coords[0, 0] = 1.0
    m_coords[1, 1] = 1.0
    m_coords[2, 2] = float(hmix_a)
    m_coords[3, 2] = float(hmix_b)
    m_coords[4, 2] = float(hmix_c)

    b1_w = np.asarray(b1_w, f); b2_w = np.asarray(b2_w, f)
    w1_f, w1_df, w1_dp = b1_w[0:W0], b1_w[W0:2 * W0], b1_w[2 * W0:]
    w2_f, w2_df, w2_dp = b2_w[0:W1], b2_w[W1:2 * W1], b2_w[2 * W1:]
    head1_w = np.asarray(head1_w, f)
    glob_w = np.asarray(glob_w, f); head2_w = np.asarray(head2_w, f)

    com = {
        "m_coords": m_coords,
        "stem_w": np.asarray(stem_w, f),
        "stem_b": np.asarray(stem_b, f).reshape(W0, 1),
        "w1_u_a": np.ascontiguousarray(w1_df),
        "w1_u_b": np.ascontiguousarray(w1_dp),
        "w1_v_a": np.ascontiguousarray(w1_f - w1_df),
        "w1_v_b": np.ascontiguousarray(-w1_dp),
        "b1_b": np.asarray(b1_b, f).reshape(W1, 1),
        "w2_u_a": np.ascontiguousarray(w2_df),
        "w2_u_b": np.ascontiguousarray(w2_dp),
        "w2_v_a": np.ascontiguousarray(w2_f - w2_df),
        "w2_v_b": np.ascontiguousarray(-w2_dp),
        "b2_b": np.ascontiguousarray(np.asarray(b2_b, f).reshape(2, 128).T),
        "glob_k0": np.ascontiguousarray(glob_w[0:128]),
        "glob_k1": np.ascontiguousarray(glob_w[128:256]),
        "glob_b": np.ascontiguousarray(np.asarray(glob_b, f).reshape(2, 128).T),
        "h1a_k0": np.ascontiguousarray(head1_w[0:128]),
        "h1a_k1": np.ascontiguousarray(head1_w[128:256]),
        "h1g_k0": np.ascontiguousarray(head1_w[256:384]),
        "h1g_k1": np.ascontiguousarray(head1_w[384:512]),
        "h1_b": np.ascontiguousarray(np.asarray(head1_b, f).reshape(2, 128).T),
        "h2_k0": np.ascontiguousarray(head2_w[0:128]),
        "h2_k1": np.ascontiguousarray(head2_w[128:256]),
        "h2_b": np.asarray(head2_b, f).reshape(NUM_CLASSES, 1),
        "sig_par": np.array([[-float(sharp), float(sharp) * float(thresh),
                              float(scale)]], f),
        "choff": np.broadcast_to(
            (np.arange(128) // 8 * 512).astype(f), (128, 128)).copy(),
    }
    in_maps = []
    for c in range(ncores):
        b, h = c // HALVES, c % HALVES
        xT = np.ascontiguousarray(x[b % B].T[:, :n])
        xT1 = np.concatenate([xT, one], 0)
        qxT1 = np.ascontiguousarray(xT1[:, h * nq:(h + 1) * nq])
        in_maps.append({"xT1": xT1, "qxT1": qxT1, **com})
    return in_maps


_CACHE = {}


def kernel(**inputs):
    from concourse.bass_utils import run_bass_kernel_spmd
    if "nc" not in _CACHE:
        _CACHE["nc"] = build_program()
    nc = _CACHE["nc"]
    in_maps = prep_inputs(**inputs)
    r = run_bass_kernel_spmd(nc, in_maps, list(range(NCORES)))
    nq = N // HALVES
    out = np.zeros((B, N, NUM_CLASSES), np.float32)
    for c in range(NCORES):
        b, h = c // HALVES, c % HALVES
        out[b, h * nq:(h + 1) * nq, :] = r.results[c]["out_lg"].T
    return out



# revision 7
# speedup vs baseline: 2165.1758x; 1.3260x over previous
"""HeightAwarePointNetTiny on 8 Trainium2 NeuronCores (Bass/Tile).

The reference LocalAggBlock computes, per point i,
    out_i = max_{j in KNN(i)} relu(W [f_i; f_j - f_i; p_j - p_i] + b).
The pre-activation separates into a j-only and an i-only part:
    u_j = W_df f_j + W_dp p_j,   v_i = (W_f - W_df) f_i - W_dp p_i + b
    out_i = relu(v_i + max_{j in KNN(i)} u_j)
so each block is two small matmuls plus a gather-max over the KNN index
lists — no k-wide MLP.  Everything runs channel-major ([C, N]) so matmul
outputs chain without transposes; neighbor gathers use GPSIMD ap_gather
over the free axis.

Sharding: core c owns cloud c//2, query half c%2 (4096 rows).  Cross-core
data: f1 (AllGather over pairs) and the global max pool (AllReduce-max).

KNN top-16 per query row: PE emits score rows s = -dist^2 into PSUM; DVE
max8/max_index/match_replace extract exact top-16 values + indices.
"""
import sys, os
sys.path.insert(0, '/opt/trn_rl_repo')
import numpy as np
from contextlib import ExitStack

import concourse.bass as bass
import concourse.tile as tile
from concourse import bacc, mybir

dt = mybir.dt
F32 = dt.float32
BF16 = dt.bfloat16

B, N, IN_CH = 4, 8192, 4
K = 16
W0, W1, W2 = 64, 128, 256
NUM_CLASSES = 3
NCORES = 8
P = 128
CH = 512                      # matmul free-dim chunk
HALVES = 2
SEL_CHUNK = int(os.environ.get("SEL_CHUNK", "0"))  # 0 = flat exact top-16


def build_program(n=N, ncores=NCORES, sel_chunk=SEL_CHUNK):
    nq = n // HALVES
    nt = nq // P
    nch = n // CH
    nqch = nq // CH
    nc = bacc.Bacc("TRN2", target_bir_lowering=False, debug=False,
                   num_devices=ncores)

    xT1 = nc.dram_tensor("xT1", [5, n], F32, kind="ExternalInput")
    qxT1 = nc.dram_tensor("qxT1", [5, nq], F32, kind="ExternalInput")
    wm = {}
    for name, shape in [
        ("m_coords", [5, 3]), ("stem_w", [4, W0]), ("stem_b", [W0, 1]),
        ("w1_u_a", [W0, W1]), ("w1_u_b", [3, W1]),
        ("w1_v_a", [W0, W1]), ("w1_v_b", [3, W1]), ("b1_b", [W1, 1]),
        ("w2_u_a", [W1, W2]), ("w2_u_b", [3, W2]),
        ("w2_v_a", [W1, W2]), ("w2_v_b", [3, W2]), ("b2_b", [128, 2]),
        ("glob_k0", [128, W2]), ("glob_k1", [128, W2]), ("glob_b", [128, 2]),
        ("h1a_k0", [128, W2]), ("h1a_k1", [128, W2]),
        ("h1g_k0", [128, W2]), ("h1g_k1", [128, W2]), ("h1_b", [128, 2]),
        ("h2_k0", [128, NUM_CLASSES]), ("h2_k1", [128, NUM_CLASSES]),
        ("h2_b", [NUM_CLASSES, 1]), ("sig_par", [1, 3]),
        ("choff", [P, P]),
    ]:
        wm[name] = nc.dram_tensor(name, shape, F32, kind="ExternalInput")

    out_lg = nc.dram_tensor("out_lg", [NUM_CLASSES, nq], F32,
                            kind="ExternalOutput")
    coords_dram = nc.dram_tensor("coords_dram", [3, n], F32)
    f1_loc = nc.dram_tensor("f1_loc", [W1, nq], F32)
    f1_gath = nc.dram_tensor("f1_gath", [HALVES, W1, nq], F32)
    g_loc = nc.dram_tensor("g_loc", [W2, 1], F32)
    g_red = nc.dram_tensor("g_red", [W2, 1], F32)
    PAIRS = [[c, c + 1] for c in range(0, ncores, 2)] if ncores > 1 else []

    Relu = mybir.ActivationFunctionType.Relu
    Copy = mybir.ActivationFunctionType.Copy
    Sigmoid = mybir.ActivationFunctionType.Sigmoid
    Square = mybir.ActivationFunctionType.Square
    AX = mybir.AxisListType.X
    MAX = mybir.AluOpType.max
    ADD = mybir.AluOpType.add

    with tile.TileContext(nc) as tc, ExitStack() as ctx:
        pers = ctx.enter_context(tc.tile_pool(name="pers", bufs=1))
        lpool = ctx.enter_context(tc.tile_pool(name="lp", bufs=2))
        gpool = ctx.enter_context(tc.tile_pool(name="gp", bufs=2))
        stg = ctx.enter_context(tc.tile_pool(name="stg", bufs=2))
        ppool = ctx.enter_context(tc.tile_pool(name="ps", bufs=4, space="PSUM"))

        def mm_chain(dst, dst_sl, parts, act=Copy, bias=0.0, scale=1.0,
                     shape=(P, CH)):
            ps = ppool.tile(list(shape), F32, tag="mm", name="mmps")
            for ix, (lhsT, rhs) in enumerate(parts):
                nc.tensor.matmul(ps[:], lhsT, rhs, start=(ix == 0),
                                 stop=(ix == len(parts) - 1))
            nc.scalar.activation(dst[:, dst_sl], ps[:], act, bias=bias,
                                 scale=scale)

        W = {}
        for name in wm:
            t = pers.tile(list(wm[name].shape), F32, tag=name, name=name)
            nc.sync.dma_start(t[:], wm[name].ap())
            W[name] = t
        wrap_all = pers.tile([P, nt * P], dt.uint16, tag="wrap_all")
        choffu = pers.tile([P, P], dt.uint16, tag="choffu")
        nc.vector.tensor_copy(choffu[:], W["choff"][:])
        MARK, MARKV, BIGNEG = float(2 ** 20), 1e30, -3e38
        MULT = mybir.AluOpType.mult
        SUB = mybir.AluOpType.subtract
        MIN = mybir.AluOpType.min
        ISEQ = mybir.AluOpType.is_equal
        ones3 = pers.tile([3, 1], F32, tag="ones3")
        nc.vector.memset(ones3[:], 1.0)
        cst = pers.tile([1, 2 * CH], F32, tag="cst")
        nc.vector.memset(cst[:, 0:CH], -1.0)
        nc.vector.memset(cst[:, CH:2 * CH], 1.0)

        with tc.tile_pool(name="poolC", bufs=1) as poolC:
            q5 = poolC.tile([5, nq], F32, tag="q5")
            f1T = poolC.tile([W1, nq], F32, tag="f1T")

            with tc.tile_pool(name="poolB", bufs=1) as poolB, \
                 tc.tile_pool(name="spool", bufs=4) as spool:
                rhs5 = poolB.tile([5, n], F32, tag="rhs5")
                U1T = poolB.tile([W1, n], F32, tag="U1T")

                # ---- streamed setup over candidate chunks ----
                for i in range(nch):
                    sl = bass.ts(i, CH)
                    xch = stg.tile([5, CH], F32, tag="xch")
                    nc.sync.dma_start(xch[:], xT1.ap()[:, sl])
                    cch = stg.tile([3, CH], F32, tag="cch")
                    ps = ppool.tile([3, CH], F32, tag="mm", name="csps")
                    nc.tensor.matmul(ps[:], W["m_coords"][:], xch[:],
                                     start=True, stop=True)
                    nc.scalar.activation(cch[:], ps[:], Copy, scale=1.0)
                    nc.scalar.activation(rhs5[0:3, sl], ps[:], Copy,
                                         scale=2.0)
                    nc.sync.dma_start(coords_dram.ap()[:, sl], cch[:])
                    sqs = stg.tile([3, CH], F32, tag="sqs")
                    nc.scalar.activation(sqs[:], cch[:], Square)
                    xxs = stg.tile([1, CH], F32, tag="xxs")
                    mm_chain(xxs, slice(0, CH), [(ones3[:], sqs[:])],
                             scale=-1.0, shape=(1, CH))
                    nc.sync.dma_start(rhs5[4:5, sl], xxs[:])
                    nc.sync.dma_start(rhs5[3:4, sl], cst[0:1, 0:CH])
                    f64 = stg.tile([W0, CH], F32, tag="f64")
                    mm_chain(f64, slice(0, CH),
                             [(W["stem_w"][:], xch[0:4, :])],
                             act=Relu, bias=W["stem_b"][:, 0:1],
                             shape=(W0, CH))
                    mm_chain(U1T, sl, [(W["w1_u_a"][:], f64[:]),
                                       (W["w1_u_b"][:], cch[:])])

                # ---- streamed setup over query chunks (V1 -> f1T) ----
                for i in range(nqch):
                    sl = bass.ts(i, CH)
                    xch = stg.tile([5, CH], F32, tag="xch")
                    nc.sync.dma_start(xch[:], qxT1.ap()[:, sl])
                    ps = ppool.tile([3, CH], F32, tag="mm", name="qcps")
                    nc.tensor.matmul(ps[:], W["m_coords"][:], xch[:],
                                     start=True, stop=True)
                    nc.scalar.activation(q5[0:3, sl], ps[:], Copy, scale=1.0)
                    sqs = stg.tile([3, CH], F32, tag="sqs")
                    nc.scalar.activation(sqs[:], ps[:], Square)
                    xxs = stg.tile([1, CH], F32, tag="xxs")
                    mm_chain(xxs, slice(0, CH), [(ones3[:], sqs[:])],
                             shape=(1, CH))
                    nc.sync.dma_start(q5[3:4, sl], xxs[:])
                    nc.sync.dma_start(q5[4:5, sl], cst[0:1, CH:2 * CH])
                    f64 = stg.tile([W0, CH], F32, tag="f64")
                    mm_chain(f64, slice(0, CH),
                             [(W["stem_w"][:], xch[0:4, :])],
                             act=Relu, bias=W["stem_b"][:, 0:1],
                             shape=(W0, CH))
                    qcch = stg.tile([3, CH], F32, tag="cch")
                    nc.scalar.activation(qcch[:], q5[0:3, sl], Copy,
                                         scale=1.0)
                    mm_chain(f1T, sl, [(W["w1_v_a"][:], f64[:]),
                                       (W["w1_v_b"][:], qcch[:])])

                # ---- loop 1: chunked 2-pass selection + block1 ----
                pend1 = []
                for t in range(nt):
                    tsl = bass.ts(t, P)
                    cand = lpool.tile([P, P], F32, tag="cand")
                    lidx = lpool.tile([P, P], dt.uint16, tag="lidx")
                    for c in range(nch):
                        ps = ppool.tile([P, CH], F32, tag="mm", name="sps")
                        nc.tensor.matmul(ps[:], q5[:, tsl],
                                         rhs5[:, bass.ts(c, CH)],
                                         start=True, stop=True)
                        srow = spool.tile([P, CH], F32, tag="srow")
                        nc.scalar.activation(srow[:], ps[:], Copy, scale=1.0)
                        c8 = slice(8 * c, 8 * c + 8)
                        nc.vector.max(cand[:, c8], srow[:])
                        nc.vector.max_index(lidx[:, c8], cand[:, c8],
                                            srow[:])
                    # global idx = local + 512*chunk; keep an f32 copy
                    nc.vector.tensor_tensor(lidx[:], lidx[:], choffu[:],
                                            op=ADD)
                    lidxf = lpool.tile([P, P], F32, tag="lidxf")
                    nc.vector.tensor_copy(lidxf[:], lidx[:])
                    # top-16 merge; winners marked in place with -MARKV
                    w16 = lpool.tile([P, K], F32, tag="w16")
                    nc.vector.max(w16[:, 0:8], cand[:])
                    nc.vector.match_replace(cand[:], w16[:, 0:8], cand[:],
                                            -MARKV)
                    nc.vector.max(w16[:, 8:16], cand[:])
                    nc.vector.match_replace(cand[:], w16[:, 8:16], cand[:],
                                            -MARKV)
                    mk = lpool.tile([P, P], F32, tag="mk")
                    nc.vector.tensor_scalar(mk[:], cand[:], -MARKV, None,
                                            op0=ISEQ)
                    pk = lpool.tile([P, P], F32, tag="pk")
                    nc.vector.scalar_tensor_tensor(pk[:], mk[:], MARK,
                                                   lidxf[:], op0=MULT,
                                                   op1=SUB)
                    gp16 = lpool.tile([P, K], F32, tag="gp16")
                    nc.vector.max(gp16[:, 0:8], pk[:])
                    nc.vector.match_replace(pk[:], gp16[:, 0:8], pk[:],
                                            BIGNEG)
                    nc.vector.max(gp16[:, 8:16], pk[:])
                    gf = lpool.tile([P, K], F32, tag="gf")
                    nc.vector.tensor_scalar(gf[:], gp16[:], -1.0, MARK,
                                            op0=MULT, op1=ADD)
                    gip = lpool.tile([P, 32], dt.uint16, tag="gip")
                    nc.vector.memset(gip[:, 16:32], 0)
                    nc.vector.tensor_scalar(gip[:, 0:16], gf[:],
                                            float(n - 1), None, op0=MIN)
                    giT = lpool.tile([32, P], dt.uint16, tag="giT")
                    for b_ in range(4):
                        nc.vector.transpose(
                            giT[0:32, 32 * b_:32 * b_ + 32],
                            gip[32 * b_:32 * b_ + 32, 0:32])
                    for g in range(8):
                        nc.sync.dma_start(wrap_all[16 * g:16 * g + 16, tsl],
                                          giT[0:16, :])

                    gat = gpool.tile([P, P * K], F32, tag="gat")
                    nc.gpsimd.ap_gather(
                        gat[:].rearrange("c (n d) -> c n d", d=1),
                        U1T[:].rearrange("c (n d) -> c n d", d=1),
                        wrap_all[:, tsl].bitcast(dt.int16),
                        channels=P, num_elems=n, d=1, num_idxs=P * K)
                    pend1.append((tsl, gat))
                    if len(pend1) > 1 or t == nt - 1:
                        for psl, pgat in pend1[:None if t == nt - 1 else -1]:
                            h1 = lpool.tile([P, P], F32, tag="h1")
                            nc.vector.tensor_reduce(
                                h1[:],
                                pgat[:].rearrange("c (q s) -> c q s", s=K),
                                axis=AX, op=MAX)
                            nc.vector.tensor_tensor(h1[:], h1[:],
                                                    f1T[:, psl], op=ADD)
                            nc.scalar.activation(f1T[:, psl], h1[:], Relu,
                                                 bias=W["b1_b"][:, 0:1],
                                                 scale=1.0)
                        pend1 = pend1[-1:] if t != nt - 1 else []

            # ---- exchange f1 halves within the pair ----
            nc.sync.dma_start(f1_loc.ap(), f1T[:])
            if PAIRS:
                nc.gpsimd.collective_compute(
                    "AllGather", mybir.AluOpType.bypass,
                    replica_groups=PAIRS,
                    ins=[f1_loc.ap()], outs=[f1_gath.ap()])
            else:   # single-core build (cost-model runs): fake the gather
                for r in range(HALVES):
                    nc.sync.dma_start(f1_gath.ap()[r], f1_loc.ap())

            with tc.tile_pool(name="poolD", bufs=1) as poolD:
                # both 128-channel halves of u2, bf16, interleaved per point
                # so one ap_gather (f32 view) fetches them together
                U2P = poolD.tile([P, 2 * n], BF16, tag="U2P")
                f2T = [poolD.tile([P, nq], F32, tag=f"f2T{o}",
                                  name=f"f2T{o}") for o in range(2)]
                U2Pv = U2P[:].rearrange("c (x h) -> c h x", h=2)
                for i in range(nch):
                    sl = bass.ts(i, CH)
                    fch = stg.tile([W1, CH], F32, tag="fch")
                    r = i // (nch // 2)
                    qoff = (i % (nch // 2)) * CH
                    nc.sync.dma_start(fch[:],
                                      f1_gath.ap()[r, :, qoff:qoff + CH])
                    cch = stg.tile([3, CH], F32, tag="cch")
                    nc.sync.dma_start(cch[:], coords_dram.ap()[:, sl])
                    for o in range(2):
                        osl = slice(o * P, o * P + P)
                        ps = ppool.tile([P, CH], F32, tag="mm", name="u2ps")
                        nc.tensor.matmul(ps[:], W["w2_u_a"][:, osl], fch[:],
                                         start=True, stop=False)
                        nc.tensor.matmul(ps[:], W["w2_u_b"][:, osl], cch[:],
                                         start=False, stop=True)
                        nc.scalar.activation(
                            U2Pv[:, o:o + 1, sl],
                            ps[:].rearrange("c (o x) -> c o x", o=1), Copy)
                # V2 -> f2T storage
                for i in range(nqch):
                    sl = bass.ts(i, CH)
                    qcch = stg.tile([3, CH], F32, tag="cch")
                    nc.scalar.activation(qcch[:], q5[0:3, sl], Copy,
                                         scale=1.0)
                    for o in range(2):
                        osl = slice(o * P, o * P + P)
                        mm_chain(f2T[o], sl,
                                 [(W["w2_v_a"][:, osl], f1T[:, sl]),
                                  (W["w2_v_b"][:, osl], qcch[:])])

                # ---- loop 2: block2 gather-max -> f2 (in place) ----
                pend2 = []
                for t in range(nt):
                    tsl = bass.ts(t, P)
                    gat2 = gpool.tile([P, P * K], F32, tag="gat2")
                    nc.gpsimd.ap_gather(
                        gat2[:].rearrange("c (n d) -> c n d", d=1),
                        U2P[:].bitcast(F32).rearrange("c (n d) -> c n d",
                                                      d=1),
                        wrap_all[:, tsl].bitcast(dt.int16),
                        channels=P, num_elems=n, d=1, num_idxs=P * K)
                    pend2.append((tsl, gat2))
                    if len(pend2) > 1 or t == nt - 1:
                        for psl, pgat in pend2[:None if t == nt - 1 else -1]:
                            gb = pgat[:].bitcast(BF16).rearrange(
                                "c (q s h) -> c q h s", s=K, h=2)
                            for o in range(2):
                                h2 = lpool.tile([P, P], F32, tag="h1")
                                nc.vector.tensor_reduce(
                                    h2[:].rearrange("c (q o) -> c q o", o=1),
                                    gb[:, :, o:o + 1, :], axis=AX, op=MAX)
                                nc.vector.tensor_tensor(h2[:], h2[:],
                                                        f2T[o][:, psl],
                                                        op=ADD)
                                nc.scalar.activation(f2T[o][:, psl], h2[:],
                                                     Relu,
                                                     bias=W["b2_b"][:,
                                                                    o:o + 1],
                                                     scale=1.0)
                        pend2 = pend2[-1:] if t != nt - 1 else []

                # ---- global max pool + glob MLP + b_eff ----
                gmx = pers.tile([P, 2], F32, tag="gmx")
                for o in range(2):
                    nc.vector.tensor_reduce(gmx[:, o:o + 1], f2T[o][:],
                                            axis=AX, op=MAX)
                    nc.sync.dma_start(g_loc.ap()[o * P:o * P + P, :],
                                      gmx[:, o:o + 1])
                if PAIRS:
                    nc.gpsimd.collective_compute(
                        "AllReduce", MAX, replica_groups=PAIRS,
                        ins=[g_loc.ap()], outs=[g_red.ap()])
                else:
                    nc.sync.dma_start(g_red.ap(), g_loc.ap())
                gsb = pers.tile([P, 2], F32, tag="gsb")
                nc.sync.dma_start(
                    gsb[:], g_red.ap().rearrange("(r p) c -> p (r c)", r=2))
                g2 = pers.tile([P, 2], F32, tag="g2")
                beff = pers.tile([P, 2], F32, tag="beff")
                for o in range(2):
                    osl = slice(o * P, o * P + P)
                    mm_chain(g2, slice(o, o + 1),
                             [(W["glob_k0"][:, osl], gsb[:, 0:1]),
                              (W["glob_k1"][:, osl], gsb[:, 1:2])],
                             act=Relu, bias=W["glob_b"][:, o:o + 1],
                             shape=(P, 1))
                for o in range(2):
                    osl = slice(o * P, o * P + P)
                    ps = ppool.tile([P, 1], F32, tag="mm", name="beffps")
                    nc.tensor.matmul(ps[:], W["h1g_k0"][:, osl], g2[:, 0:1],
                                     start=True, stop=False)
                    nc.tensor.matmul(ps[:], W["h1g_k1"][:, osl], g2[:, 1:2],
                                     start=False, stop=True)
                    nc.vector.tensor_scalar(beff[:, o:o + 1], ps[:],
                                            W["h1_b"][:, o:o + 1], None,
                                            op0=ADD)

                # ---- loop 3: head ----
                for t in range(nt):
                    tsl = bass.ts(t, P)
                    hT = [lpool.tile([P, P], F32, tag=f"hT{o}",
                                     name=f"hT{o}") for o in range(2)]
                    for o in range(2):
                        osl = slice(o * P, o * P + P)
                        ps = ppool.tile([P, P], F32, tag="mm", name="hps")
                        nc.tensor.matmul(ps[:], W["h1a_k0"][:, osl],
                                         f2T[0][:, tsl],
                                         start=True, stop=False)
                        nc.tensor.matmul(ps[:], W["h1a_k1"][:, osl],
                                         f2T[1][:, tsl],
                                         start=False, stop=True)
                        nc.scalar.activation(hT[o][:], ps[:], Relu,
                                             bias=beff[:, o:o + 1],
                                             scale=1.0)
                    ps3 = ppool.tile([NUM_CLASSES, P], F32, tag="mm",
                                     name="lps")
                    nc.tensor.matmul(ps3[:], W["h2_k0"][:], hT[0][:],
                                     start=True, stop=False)
                    nc.tensor.matmul(ps3[:], W["h2_k1"][:], hT[1][:],
                                     start=False, stop=True)
                    lg = lpool.tile([NUM_CLASSES, P], F32, tag="lg")
                    nc.vector.tensor_scalar(lg[:], ps3[:],
                                            W["h2_b"][:, 0:1], None,
                                            op0=ADD)
                    sg = lpool.tile([1, P], F32, tag="sg")
                    hagt = lpool.tile([1, P], F32, tag="hagt")
                    nc.sync.dma_start(hagt[:], qxT1.ap()[3:4, tsl])
                    # sig_par: [-sharp, sharp*thresh, scale]
                    nc.scalar.activation(sg[:], hagt[:], Sigmoid,
                                         bias=W["sig_par"][0:1, 1:2],
                                         scale=W["sig_par"][0:1, 0:1])
                    nc.vector.scalar_tensor_tensor(
                        lg[0:1, :], sg[:], W["sig_par"][0:1, 2:3],
                        lg[0:1, :], op0=mybir.AluOpType.mult, op1=ADD)
                    nc.sync.dma_start(out_lg.ap()[:, tsl], lg[:])

    nc.compile()
    return nc


def prep_inputs(x, hmix_a, hmix_b, hmix_c, stem_w, stem_b, b1_w, b1_b,
                b2_w, b2_b, glob_w, glob_b, head1_w, head1_b,
                head2_w, head2_b, thresh, sharp, scale, n=N, ncores=NCORES):
    """Host-side layout prep: per-core input maps (data movement + weight
    repacking only)."""
    f = np.float32
    nq = n // HALVES
    x = np.asarray(x, f)
    one = np.ones((1, n), f)
    m_coords = np.zeros((5, 3), f)
    m_coords[0, 0] = 1.0
    m_coords[1, 1] = 1.0
    m_coords[2, 2] = float(hmix_a)
    m_coords[3, 2] = float(hmix_b)
    m_coords[4, 2] = float(hmix_c)

    b1_w = np.asarray(b1_w, f); b2_w = np.asarray(b2_w, f)
    w1_f, w1_df, w1_dp = b1_w[0:W0], b1_w[W0:2 * W0], b1_w[2 * W0:]
    w2_f, w2_df, w2_dp = b2_w[0:W1], b2_w[W1:2 * W1], b2_w[2 * W1:]
    head1_w = np.asarray(head1_w, f)
    glob_w = np.asarray(glob_w, f); head2_w = np.asarray(head2_w, f)

    com = {
        "m_coords": m_coords,
        "stem_w": np.asarray(stem_w, f),
        "stem_b": np.asarray(stem_b, f).reshape(W0, 1),
        "w1_u_a": np.ascontiguousarray(w1_df),
        "w1_u_b": np.ascontiguousarray(w1_dp),
        "w1_v_a": np.ascontiguousarray(w1_f - w1_df),
        "w1_v_b": np.ascontiguousarray(-w1_dp),
        "b1_b": np.asarray(b1_b, f).reshape(W1, 1),
        "w2_u_a": np.ascontiguousarray(w2_df),
        "w2_u_b": np.ascontiguousarray(w2_dp),
        "w2_v_a": np.ascontiguousarray(w2_f - w2_df),
        "w2_v_b": np.ascontiguousarray(-w2_dp),
        "b2_b": np.ascontiguousarray(np.asarray(b2_b, f).reshape(2, 128).T),
        "glob_k0": np.ascontiguousarray(glob_w[0:128]),
        "glob_k1": np.ascontiguousarray(glob_w[128:256]),
        "glob_b": np.ascontiguousarray(np.asarray(glob_b, f).reshape(2, 128).T),
        "h1a_k0": np.ascontiguousarray(head1_w[0:128]),
        "h1a_k1": np.ascontiguousarray(head1_w[128:256]),
        "h1g_k0": np.ascontiguousarray(head1_w[256:384]),
        "h1g_k1": np.ascontiguousarray(head1_w[384:512]),
        "h1_b": np.ascontiguousarray(np.asarray(head1_b, f).reshape(2, 128).T),
        "h2_k0": np.ascontiguousarray(head2_w[0:128]),
        "h2_k1": np.ascontiguousarray(head2_w[128:256]),
        "h2_b": np.asarray(head2_b, f).reshape(NUM_CLASSES, 1),
        "sig_par": np.array([[-float(sharp), float(sharp) * float(thresh),
                              float(scale)]], f),
        "choff": np.broadcast_to(
            (np.arange(128) // 8 * 512).astype(f), (128, 128)).copy(),
    }
    in_maps = []
    for c in range(ncores):
        b, h = c // HALVES, c % HALVES
        xT = np.ascontiguousarray(x[b % B].T[:, :n])
        xT1 = np.concatenate([xT, one], 0)
        qxT1 = np.ascontiguousarray(xT1[:, h * nq:(h + 1) * nq])
        in_maps.append({"xT1": xT1, "qxT1": qxT1, **com})
    return in_maps


_CACHE = {}


def kernel(**inputs):
    from concourse.bass_utils import run_bass_kernel_spmd
    if "nc" not in _CACHE:
        _CACHE["nc"] = build_program()
    nc = _CACHE["nc"]
    in_maps = prep_inputs(**inputs)
    r = run_bass_kernel_spmd(nc, in_maps, list(range(NCORES)))
    nq = N // HALVES
    out = np.zeros((B, N, NUM_CLASSES), np.float32)
    for c in range(NCORES):
        b, h = c // HALVES, c % HALVES
        out[b, h * nq:(h + 1) * nq, :] = r.results[c]["out_lg"].T
    return out

